# revision 38
# baseline (speedup 1.0000x reference)
"""Multi-head attention (B=2, S=2048, D=1024, H=16, causal mask) on 8 TRN2 cores.

Sharding: core c handles batch b = c//4 and 4 heads g = c%4 (dims 256g..256g+256
of the projection space).  Each core computes a partial output [S, D] (its 4
heads' contribution to the out-projection); the host sums the 4 partials per
batch and adds the output bias.

Device layout (per core) keeps the sequence axis on the SBUF free dimension:
  QT, KT  [256, 2048]  (head-dim on partitions, 2 head-pairs of 128)
  V_aug   16 tiles [128, 4, 65]  (seq on partitions; per head 64 dims + ones col)
  scores  S.T tiles [128 k, 512 q] per head; causal blocks above diagonal skipped
  exp     ScalarE, scale=1/8, mask folded in as a -1e9 bias (one [128,128] tri tile)
  ctx.T   [65, 512] PSUM per (head, q-chunk); row 64 = softmax denominator l
  norm    reciprocal_approx_fast on l, partition_broadcast, DVE multiply
  out     ctxT (4 heads stacked, [256, 2048]) @ o_w slice -> [2048, 1024]
All matmuls run as float32r (f32 storage bitcast; full PE rate at N>=256).
"""

import numpy as np
from contextlib import ExitStack

import concourse.bacc as bacc
import concourse.bass as bass
import concourse.tile as tile
from concourse import mybir

P = 128
S = 2048
D = 1024
N_HEADS_TOT = 16
HEADS = 4            # per core
HD = 64
M_DIM = HEADS * HD   # 256
KC = 8               # embed-dim 128-chunks
QCW = 512            # q chunk width
NQC = S // QCW       # 4
NKT = S // P         # 16 k-tiles
F32 = mybir.dt.float32
F32R = mybir.dt.float32r
EXPF = mybir.ActivationFunctionType.Exp
NEG = -1.0e9

TRACE = False
LAST_RESULTS = None
_NC_CACHE = {}


def build_nc(mode: str, compile_: bool = True, probes: bool = False,
             has_bias: bool = False) -> bass.Bass:
    """mode in {causal, nomask, generic}"""
    nc = bacc.Bacc("TRN2", target_bir_lowering=False, debug=False)
    prb = {}
    if probes:
        for nm, shape in (("p_qt", [P, S]), ("p_kt", [P, S]), ("p_va", [P, HEADS * (HD + 1)]),
                          ("p_s", [P, QCW]), ("p_p", [P, QCW]), ("p_ctx", [HD + 1, QCW]),
                          ("p_r", [1, QCW]), ("p_rbc", [HD, QCW]), ("p_ct", [P, S])):
            prb[nm] = nc.dram_tensor(nm, shape, F32, kind="ExternalOutput").ap()
    xq = nc.dram_tensor("xqT", [D + 1, S], F32R, kind="ExternalInput").ap()
    xk = nc.dram_tensor("xkT", [D + 1, S], F32R, kind="ExternalInput").ap()
    xv = nc.dram_tensor("xvT", [D + 1, S], F32R, kind="ExternalInput").ap()
    wq = nc.dram_tensor("wqT", [D + 1, M_DIM], F32R, kind="ExternalInput").ap()
    wk = nc.dram_tensor("wkT", [D + 1, M_DIM], F32R, kind="ExternalInput").ap()
    wv = nc.dram_tensor("wvT", [D + 1, M_DIM], F32R, kind="ExternalInput").ap()
    ow = nc.dram_tensor("owT", [M_DIM, D], F32R, kind="ExternalInput").ap()
    btri = nc.dram_tensor("btri", [P, P], F32, kind="ExternalInput").ap()
    bfull = None
    if mode == "generic":
        bfull = nc.dram_tensor("biasT", [S, S], F32, kind="ExternalInput").ap()
    out = nc.dram_tensor("out", [S, D], F32, kind="ExternalOutput").ap()

    with tile.TileContext(nc) as tc, ExitStack() as ctx:
        consts = ctx.enter_context(tc.tile_pool(name="consts", bufs=1))
        xpool = ctx.enter_context(tc.tile_pool(name="xpool", bufs=8))
        qkv = ctx.enter_context(tc.tile_pool(name="qkv", bufs=1))
        ppool = ctx.enter_context(tc.tile_pool(name="ppool", bufs=4))
        bpool = ctx.enter_context(tc.tile_pool(name="bpool", bufs=2))
        small = ctx.enter_context(tc.tile_pool(name="small", bufs=4))
        outp = ctx.enter_context(tc.tile_pool(name="outp", bufs=1 if probes else 2))
        spool = ctx.enter_context(tc.tile_pool(name="spsum", bufs=3, space="PSUM"))
        cpool = ctx.enter_context(tc.tile_pool(name="cpsum", bufs=2, space="PSUM"))
        opool = ctx.enter_context(tc.tile_pool(name="opsum", bufs=2, space="PSUM"))

        # ---- resident weights ----
        def load_w(ap_dram, nm):
            tiles = []
            for kc in range(KC):
                t = consts.tile([P, M_DIM], F32R, name=f"{nm}{kc}")
                nc.sync.dma_start(out=t, in_=ap_dram[P * kc:P * (kc + 1), :])
                tiles.append(t)
            aug = None
            if has_bias:
                aug = consts.tile([1, M_DIM], F32R, name=f"{nm}_aug")
                nc.sync.dma_start(out=aug, in_=ap_dram[D:D + 1, :])
            return tiles, aug

        wq_sb, wq_aug = load_w(wq, "wq")
        wk_sb, wk_aug = load_w(wk, "wk")
        wv_sb, wv_aug = load_w(wv, "wv")
        ow_sb = []
        for pr in range(2):
            t = consts.tile([P, D], F32R, name=f"ow{pr}")
            nc.sync.dma_start(out=t, in_=ow[P * pr:P * (pr + 1), :])
            ow_sb.append(t)
        btri_sb = consts.tile([P, P], F32, name="btri_sb")
        nc.sync.dma_start(out=btri_sb, in_=btri)
        ones4 = consts.tile([P, HEADS], F32, name="ones4")
        nc.vector.memset(ones4, 1.0)

        QT = [qkv.tile([P, S], F32R, name=f"QT{pr}") for pr in range(2)]
        KT = [qkv.tile([P, S], F32R, name=f"KT{pr}") for pr in range(2)]
        CT = [qkv.tile([P, S], F32R, name=f"CT{pr}") for pr in range(2)]
        VA = [qkv.tile([P, HEADS, HD + 1], F32R, name=f"VA{t}") for t in range(NKT)]

        # ---- Q/K projections:  dest[pair][:, n] = w.T @ xT  ----
        for xap, w_sb, w_aug, dest in ((xq, wq_sb, wq_aug, QT), (xk, wk_sb, wk_aug, KT)):
            x_sb = []
            for kc in range(KC):
                xt = xpool.tile([P, S], F32R, name="xt")
                nc.sync.dma_start(out=xt, in_=xap[P * kc:P * (kc + 1), :])
                x_sb.append(xt)
            xaug = None
            if has_bias:
                xaug = small.tile([1, S], F32R, name="xaug", bufs=2)
                nc.sync.dma_start(out=xaug, in_=xap[D:D + 1, :])
            for m in range(2):
                for n in range(NQC):
                    ps = spool.tile([P, QCW], F32, name="s_ps")
                    for kc in range(KC):
                        nc.tensor.matmul(
                            ps,
                            lhsT=w_sb[kc][:, P * m:P * (m + 1)],
                            rhs=x_sb[kc][:, QCW * n:QCW * (n + 1)],
                            start=(kc == 0),
                            stop=(not has_bias and kc == KC - 1))
                    if has_bias:
                        nc.tensor.matmul(
                            ps,
                            lhsT=w_aug[0:1, P * m:P * (m + 1)],
                            rhs=xaug[0:1, QCW * n:QCW * (n + 1)],
                            start=False, stop=True)
                    nc.vector.tensor_copy(dest[m][:, QCW * n:QCW * (n + 1)], ps)

        # ---- V projection (natural layout + ones column) ----
        xv_sb = []
        for kc in range(KC):
            xt = xpool.tile([P, S], F32R, name="xt")
            nc.sync.dma_start(out=xt, in_=xv[P * kc:P * (kc + 1), :])
            xv_sb.append(xt)
        xv_aug = None
        if has_bias:
            xv_aug = small.tile([1, S], F32R, name="xaug", bufs=2)
            nc.sync.dma_start(out=xv_aug, in_=xv[D:D + 1, :])
        for m in range(NKT):
            ps = spool.tile([P, QCW], F32, name="s_ps")
            for kc in range(KC):
                nc.tensor.matmul(
                    ps[:, 0:M_DIM],
                    lhsT=xv_sb[kc][:, P * m:P * (m + 1)],
                    rhs=wv_sb[kc],
                    start=(kc == 0),
                    stop=(not has_bias and kc == KC - 1))
            if has_bias:
                nc.tensor.matmul(
                    ps[:, 0:M_DIM],
                    lhsT=xv_aug[0:1, P * m:P * (m + 1)],
                    rhs=wv_aug,
                    start=False, stop=True)
            nc.vector.tensor_copy(
                VA[m][:, :, 0:HD],
                ps[:, 0:M_DIM].rearrange("p (h d) -> p h d", h=HEADS))
            nc.vector.tensor_copy(
                VA[m][:, :, HD:HD + 1],
                ones4.rearrange("p (h o) -> p h o", o=1))

        if probes:
            nc.sync.dma_start(out=prb["p_qt"], in_=QT[0].bitcast(F32))
            nc.sync.dma_start(out=prb["p_kt"], in_=KT[0].bitcast(F32))
            nc.sync.dma_start(
                out=prb["p_va"],
                in_=VA[0].rearrange("p h d -> p (h d)").bitcast(F32))

        # ---- attention + out-projection ----
        for qc in range(NQC):
            for h in range(HEADS):
                pr, off = divmod(h, 2)
                ctx_ps = cpool.tile([HD + 1, QCW], F32, name="ctx_ps")
                nt = 4 * qc + 4 if mode == "causal" else NKT
                for t in range(nt):
                    o = max(0, P * t - QCW * qc) if mode == "causal" else 0
                    s_ps = spool.tile([P, QCW], F32, name="s_ps")
                    nc.tensor.matmul(
                        s_ps[:, o:],
                        lhsT=KT[pr][HD * off:HD * (off + 1), P * t:P * (t + 1)],
                        rhs=QT[pr][HD * off:HD * (off + 1), QCW * qc + o:QCW * (qc + 1)],
                        start=True, stop=True)
                    if mode == "causal" and t >= 4 * qc:
                        nc.vector.tensor_add(
                            s_ps[:, o:o + P], s_ps[:, o:o + P], btri_sb)
                    elif mode == "generic":
                        bt = bpool.tile([P, QCW], F32, name="bt")
                        nc.sync.dma_start(
                            out=bt,
                            in_=bfull[P * t:P * (t + 1), QCW * qc:QCW * (qc + 1)])
                        nc.vector.tensor_add(s_ps, s_ps, bt)
                    if probes and qc == 0 and h == 0 and t == 0:
                        dbg = ppool.tile([P, QCW], F32, name="dbg", bufs=1)
                        nc.vector.tensor_copy(dbg, s_ps)
                        nc.sync.dma_start(out=prb["p_s"], in_=dbg)
                    p_sb = ppool.tile([P, QCW], F32R, name="p_sb")
                    nc.scalar.activation(p_sb[:, o:], s_ps[:, o:], EXPF, scale=0.125)
                    if probes and qc == 0 and h == 0 and t == 0:
                        nc.sync.dma_start(out=prb["p_p"], in_=p_sb.bitcast(F32))
                    nc.tensor.matmul(
                        ctx_ps[:, o:],
                        lhsT=VA[t][:, h, :],
                        rhs=p_sb[:, o:],
                        start=(t == 0), stop=(t == nt - 1),
                        skip_group_check=True)
                if probes and qc == 0 and h == 0:
                    dbg2 = ppool.tile([HD + 1, QCW], F32, name="dbg", bufs=1)
                    nc.vector.tensor_copy(dbg2, ctx_ps)
                    nc.sync.dma_start(out=prb["p_ctx"], in_=dbg2)
                l_sb = small.tile([1, QCW], F32, name="l_sb", bufs=3)
                nc.vector.tensor_copy(l_sb, ctx_ps[HD:HD + 1, :])
                r_sb = small.tile([1, QCW], F32, name="r_sb", bufs=3)
                nc.vector.reciprocal_approx_fast(out=r_sb, in_=l_sb)
                rbc = ppool.tile([HD, QCW], F32, name="rbc", bufs=2)
                nc.gpsimd.partition_broadcast(out_ap=rbc, in_ap=r_sb)
                if probes and qc == 0 and h == 0:
                    nc.sync.dma_start(out=prb["p_r"], in_=r_sb)
                    nc.sync.dma_start(out=prb["p_rbc"], in_=rbc)
                nc.vector.tensor_mul(
                    CT[pr][HD * off:HD * (off + 1), QCW * qc:QCW * (qc + 1)],
                    ctx_ps[0:HD, :], rbc)
            for mq in range(QCW // P):
                out_sb = outp.tile([P, D], F32, name="out_sb")
                q0 = QCW * qc + P * mq
                for ne in range(2):
                    o_ps = opool.tile([P, QCW], F32, name="o_ps")
                    for pr2 in range(2):
                        nc.tensor.matmul(
                            o_ps,
                            lhsT=CT[pr2][:, q0:q0 + P],
                            rhs=ow_sb[pr2][:, QCW * ne:QCW * (ne + 1)],
                            start=(pr2 == 0), stop=(pr2 == 1))
                    nc.vector.tensor_copy(out_sb[:, QCW * ne:QCW * (ne + 1)], o_ps)
                nc.sync.dma_start(out=out[q0:q0 + P, :], in_=out_sb)
        if probes:
            nc.sync.dma_start(out=prb["p_ct"], in_=CT[0].bitcast(F32))

    if compile_:
        nc.compile()
    return nc


def _get_nc(mode, has_bias):
    key = (mode, has_bias)
    if key not in _NC_CACHE:
        _NC_CACHE[key] = build_nc(mode, has_bias=has_bias)
    return _NC_CACHE[key]


def _tri_bias():
    g = np.arange(P, dtype=np.int64)
    return np.where(g[None, :] < g[:, None], np.float32(NEG), np.float32(0.0))


def host_prep(query, key, value, attn_mask, q_w, q_b, k_w, k_b, v_w, v_b, o_w, o_b):
    """Build (mode, in_maps) for the 8 cores."""
    mask = np.asarray(attn_mask).astype(bool)
    if np.array_equal(mask, np.triu(np.ones((S, S), bool), 1)):
        mode = "causal"
    elif not mask.any():
        mode = "nomask"
    else:
        mode = "generic"

    ones_row = np.ones((1, S), np.float32)

    def prep_x(x):
        return np.vstack([np.ascontiguousarray(x.T, dtype=np.float32), ones_row])

    xs = {}
    for b in range(2):
        xs[b] = (prep_x(np.asarray(query)[b]), prep_x(np.asarray(key)[b]),
                 prep_x(np.asarray(value)[b]))

    tri = _tri_bias()
    biasT = None
    if mode == "generic":
        biasT = np.ascontiguousarray(
            np.where(mask, np.float32(NEG), np.float32(0.0)).T)

    def prep_w(w, bvec, sl):
        return np.vstack([
            np.ascontiguousarray(np.asarray(w)[sl].T, dtype=np.float32),
            np.asarray(bvec)[sl][None, :].astype(np.float32)])

    in_maps = []
    for c in range(8):
        b, g = divmod(c, 4)
        sl = slice(M_DIM * g, M_DIM * (g + 1))
        m = {
            "xqT": xs[b][0], "xkT": xs[b][1], "xvT": xs[b][2],
            "wqT": prep_w(q_w, q_b, sl),
            "wkT": prep_w(k_w, k_b, sl),
            "wvT": prep_w(v_w, v_b, sl),
            "owT": np.ascontiguousarray(np.asarray(o_w)[:, sl].T, dtype=np.float32),
            "btri": tri,
        }
        if mode == "generic":
            m["biasT"] = biasT
        in_maps.append(m)
    return mode, in_maps


def kernel(**inputs) -> np.ndarray:
    global LAST_RESULTS
    from concourse.bass_utils import run_bass_kernel_spmd

    mode, in_maps = host_prep(**inputs)
    has_bias = any(
        np.asarray(inputs[k]).any() for k in ("q_b", "k_b", "v_b"))
    nc = _get_nc(mode, has_bias)
    res = run_bass_kernel_spmd(nc, in_maps, core_ids=list(range(8)), trace=TRACE)
    LAST_RESULTS = res
    parts = [res.results[c]["out"] for c in range(8)]
    o_b = np.asarray(inputs["o_b"]).astype(np.float32)
    out = np.stack([
        parts[0] + parts[1] + parts[2] + parts[3],
        parts[4] + parts[5] + parts[6] + parts[7],
    ], axis=0) + o_b[None, None, :]
    return out.astype(np.float32)


# revision 41
# speedup vs baseline: 1.0827x; 1.0827x over previous
"""Multi-head attention (B=2, S=2048, D=1024, H=16, causal mask) on 8 TRN2 cores.

Sharding: core c handles batch b = c//4 and 4 heads g = c%4 (dims 256g..256g+256
of the projection space).  Each core computes a partial output [S, D] (its 4
heads' contribution to the out-projection); the host sums the 4 partials per
batch and adds the output bias.

Device layout (per core) keeps the sequence axis on the SBUF free dimension:
  QT, KT  [256, 2048]  (head-dim on partitions, 2 head-pairs of 128)
  V_aug   16 tiles [128, 4, 65]  (seq on partitions; per head 64 dims + ones col)
  scores  S.T tiles [128 k, 512 q] per head; causal blocks above diagonal skipped
  exp     ScalarE, scale=1/8, mask folded in as a -1e9 bias (one [128,128] tri tile)
  ctx.T   [65, 512] PSUM per (head, q-chunk); row 64 = softmax denominator l
  norm    reciprocal_approx_fast on l, partition_broadcast, DVE multiply
  out     ctxT (4 heads stacked, [256, 2048]) @ o_w slice -> [2048, 1024]
All matmuls run as float32r (f32 storage bitcast; full PE rate at N>=256).
"""

import numpy as np
from contextlib import ExitStack

import concourse.bacc as bacc
import concourse.bass as bass
import concourse.tile as tile
from concourse import mybir

P = 128
S = 2048
D = 1024
N_HEADS_TOT = 16
HEADS = 4            # per core
HD = 64
M_DIM = HEADS * HD   # 256
KC = 8               # embed-dim 128-chunks
QCW = 512            # q chunk width
NQC = S // QCW       # 4
NKT = S // P         # 16 k-tiles
F32 = mybir.dt.float32
F32R = mybir.dt.float32r
BF16 = mybir.dt.bfloat16
EXPF = mybir.ActivationFunctionType.Exp
NEG = -1.0e9

TRACE = False
LAST_RESULTS = None
_NC_CACHE = {}


def build_nc(mode: str, compile_: bool = True, probes: bool = False,
             has_bias: bool = False) -> bass.Bass:
    """mode in {causal, nomask, generic}"""
    nc = bacc.Bacc("TRN2", target_bir_lowering=False, debug=False)
    prb = {}
    if probes:
        for nm, shape in (("p_qt", [P, S]), ("p_kt", [P, S]), ("p_va", [P, HEADS * (HD + 1)]),
                          ("p_s", [P, QCW]), ("p_p", [P, QCW]), ("p_ctx", [HD + 1, QCW]),
                          ("p_r", [1, QCW]), ("p_rbc", [HD, QCW]), ("p_ct", [P, S])):
            prb[nm] = nc.dram_tensor(nm, shape, F32, kind="ExternalOutput").ap()
    xq = nc.dram_tensor("xqT", [D + 1, S], BF16, kind="ExternalInput").ap()
    xk = nc.dram_tensor("xkT", [D + 1, S], BF16, kind="ExternalInput").ap()
    xv = nc.dram_tensor("xvT", [D + 1, S], BF16, kind="ExternalInput").ap()
    wq = nc.dram_tensor("wqT", [D + 1, M_DIM], BF16, kind="ExternalInput").ap()
    wk = nc.dram_tensor("wkT", [D + 1, M_DIM], BF16, kind="ExternalInput").ap()
    wv = nc.dram_tensor("wvT", [D + 1, M_DIM], BF16, kind="ExternalInput").ap()
    ow = nc.dram_tensor("owT", [M_DIM, D], F32R, kind="ExternalInput").ap()
    btri = nc.dram_tensor("btri", [P, P], F32, kind="ExternalInput").ap()
    bfull = None
    if mode == "generic":
        bfull = nc.dram_tensor("biasT", [S, S], F32, kind="ExternalInput").ap()
    out = nc.dram_tensor("out", [S, D], F32, kind="ExternalOutput").ap()

    with tile.TileContext(nc) as tc, ExitStack() as ctx:
        consts = ctx.enter_context(tc.tile_pool(name="consts", bufs=1))
        xpool = ctx.enter_context(tc.tile_pool(name="xpool", bufs=8))
        qkv = ctx.enter_context(tc.tile_pool(name="qkv", bufs=1))
        ppool = ctx.enter_context(tc.tile_pool(name="ppool", bufs=4))
        bpool = ctx.enter_context(tc.tile_pool(name="bpool", bufs=2))
        small = ctx.enter_context(tc.tile_pool(name="small", bufs=4))
        outp = ctx.enter_context(tc.tile_pool(name="outp", bufs=1 if probes else 2))
        spool = ctx.enter_context(tc.tile_pool(name="spsum", bufs=4, space="PSUM"))
        cpool = ctx.enter_context(tc.tile_pool(name="cpsum", bufs=2, space="PSUM"))
        opool = ctx.enter_context(tc.tile_pool(name="opsum", bufs=2, space="PSUM"))

        # ---- resident weights ----
        def load_w(ap_dram, nm):
            tiles = []
            for kc in range(KC):
                t = consts.tile([P, M_DIM], BF16, name=f"{nm}{kc}")
                nc.sync.dma_start(out=t, in_=ap_dram[P * kc:P * (kc + 1), :])
                tiles.append(t)
            aug = None
            if has_bias:
                aug = consts.tile([1, M_DIM], BF16, name=f"{nm}_aug")
                nc.sync.dma_start(out=aug, in_=ap_dram[D:D + 1, :])
            return tiles, aug

        wq_sb, wq_aug = load_w(wq, "wq")
        wk_sb, wk_aug = load_w(wk, "wk")
        wv_sb, wv_aug = load_w(wv, "wv")
        ow_sb = []
        for pr in range(2):
            t = consts.tile([P, D], F32R, name=f"ow{pr}")
            nc.sync.dma_start(out=t, in_=ow[P * pr:P * (pr + 1), :])
            ow_sb.append(t)
        btri_sb = consts.tile([P, P], F32, name="btri_sb")
        nc.sync.dma_start(out=btri_sb, in_=btri)
        ones4 = consts.tile([P, HEADS], F32, name="ones4")
        nc.vector.memset(ones4, 1.0)

        QT = [qkv.tile([P, S], F32R, name=f"QT{pr}") for pr in range(2)]
        KT = [qkv.tile([P, S], F32R, name=f"KT{pr}") for pr in range(2)]
        CT = [qkv.tile([P, S], F32R, name=f"CT{pr}") for pr in range(2)]
        VA = [qkv.tile([P, HEADS, HD + 1], F32R, name=f"VA{t}") for t in range(NKT)]

        # ---- Q/K projections:  dest[pair][:, n] = w.T @ xT  ----
        for xap, w_sb, w_aug, dest in ((xq, wq_sb, wq_aug, QT), (xk, wk_sb, wk_aug, KT)):
            x_sb = []
            for kc in range(KC):
                xt = xpool.tile([P, S], BF16, name="xt")
                nc.sync.dma_start(out=xt, in_=xap[P * kc:P * (kc + 1), :])
                x_sb.append(xt)
            xaug = None
            if has_bias:
                xaug = small.tile([1, S], BF16, name="xaug", bufs=2)
                nc.sync.dma_start(out=xaug, in_=xap[D:D + 1, :])
            for m in range(2):
                for n in range(NQC):
                    ps = spool.tile([P, QCW], F32, name="s_ps")
                    for kc in range(KC):
                        nc.tensor.matmul(
                            ps,
                            lhsT=w_sb[kc][:, P * m:P * (m + 1)],
                            rhs=x_sb[kc][:, QCW * n:QCW * (n + 1)],
                            start=(kc == 0),
                            stop=(not has_bias and kc == KC - 1))
                    if has_bias:
                        nc.tensor.matmul(
                            ps,
                            lhsT=w_aug[0:1, P * m:P * (m + 1)],
                            rhs=xaug[0:1, QCW * n:QCW * (n + 1)],
                            start=False, stop=True)
                    nc.vector.tensor_copy(dest[m][:, QCW * n:QCW * (n + 1)], ps)

        # ---- V projection (natural layout + ones column) ----
        xv_sb = []
        for kc in range(KC):
            xt = xpool.tile([P, S], BF16, name="xt")
            nc.sync.dma_start(out=xt, in_=xv[P * kc:P * (kc + 1), :])
            xv_sb.append(xt)
        xv_aug = None
        if has_bias:
            xv_aug = small.tile([1, S], BF16, name="xaug", bufs=2)
            nc.sync.dma_start(out=xv_aug, in_=xv[D:D + 1, :])
        for m in range(NKT):
            ps = spool.tile([P, QCW], F32, name="s_ps")
            for kc in range(KC):
                nc.tensor.matmul(
                    ps[:, 0:M_DIM],
                    lhsT=xv_sb[kc][:, P * m:P * (m + 1)],
                    rhs=wv_sb[kc],
                    start=(kc == 0),
                    stop=(not has_bias and kc == KC - 1))
            if has_bias:
                nc.tensor.matmul(
                    ps[:, 0:M_DIM],
                    lhsT=xv_aug[0:1, P * m:P * (m + 1)],
                    rhs=wv_aug,
                    start=False, stop=True)
            nc.vector.tensor_copy(
                VA[m][:, :, 0:HD],
                ps[:, 0:M_DIM].rearrange("p (h d) -> p h d", h=HEADS))
            nc.vector.tensor_copy(
                VA[m][:, :, HD:HD + 1],
                ones4.rearrange("p (h o) -> p h o", o=1))

        if probes:
            nc.sync.dma_start(out=prb["p_qt"], in_=QT[0].bitcast(F32))
            nc.sync.dma_start(out=prb["p_kt"], in_=KT[0].bitcast(F32))
            nc.sync.dma_start(
                out=prb["p_va"],
                in_=VA[0].rearrange("p h d -> p (h d)").bitcast(F32))

        # ---- attention + out-projection ----
        for qc in range(NQC):
            for h in range(HEADS):
                pr, off = divmod(h, 2)
                ctx_ps = cpool.tile([HD + 1, QCW], F32, name="ctx_ps")
                nt = 4 * qc + 4 if mode == "causal" else NKT
                queue = []

                def flush_ctx():
                    t0, p0, o0 = queue.pop(0)
                    nc.tensor.matmul(
                        ctx_ps[:, o0:],
                        lhsT=VA[t0][:, h, :],
                        rhs=p0[:, o0:],
                        start=(t0 == 0), stop=(t0 == nt - 1),
                        skip_group_check=True)

                for t in range(nt):
                    o = max(0, P * t - QCW * qc) if mode == "causal" else 0
                    s_ps = spool.tile([P, QCW], F32, name="s_ps")
                    nc.tensor.matmul(
                        s_ps[:, o:],
                        lhsT=KT[pr][HD * off:HD * (off + 1), P * t:P * (t + 1)],
                        rhs=QT[pr][HD * off:HD * (off + 1), QCW * qc + o:QCW * (qc + 1)],
                        start=True, stop=True)
                    if mode == "causal" and t >= 4 * qc:
                        nc.vector.tensor_add(
                            s_ps[:, o:o + P], s_ps[:, o:o + P], btri_sb)
                    elif mode == "generic":
                        bt = bpool.tile([P, QCW], F32, name="bt")
                        nc.sync.dma_start(
                            out=bt,
                            in_=bfull[P * t:P * (t + 1), QCW * qc:QCW * (qc + 1)])
                        nc.vector.tensor_add(s_ps, s_ps, bt)
                    if probes and qc == 0 and h == 0 and t == 0:
                        dbg = ppool.tile([P, QCW], F32, name="dbg", bufs=1)
                        nc.vector.tensor_copy(dbg, s_ps)
                        nc.sync.dma_start(out=prb["p_s"], in_=dbg)
                    p_sb = ppool.tile([P, QCW], F32R, name="p_sb")
                    nc.scalar.activation(p_sb[:, o:], s_ps[:, o:], EXPF, scale=0.125)
                    if probes and qc == 0 and h == 0 and t == 0:
                        nc.sync.dma_start(out=prb["p_p"], in_=p_sb.bitcast(F32))
                    queue.append((t, p_sb, o))
                    if len(queue) > 2:
                        flush_ctx()
                while queue:
                    flush_ctx()
                if probes and qc == 0 and h == 0:
                    dbg2 = ppool.tile([HD + 1, QCW], F32, name="dbg", bufs=1)
                    nc.vector.tensor_copy(dbg2, ctx_ps)
                    nc.sync.dma_start(out=prb["p_ctx"], in_=dbg2)
                l_sb = small.tile([1, QCW], F32, name="l_sb", bufs=3)
                nc.vector.tensor_copy(l_sb, ctx_ps[HD:HD + 1, :])
                r_sb = small.tile([1, QCW], F32, name="r_sb", bufs=3)
                nc.vector.reciprocal_approx_fast(out=r_sb, in_=l_sb)
                rbc = ppool.tile([HD, QCW], F32, name="rbc", bufs=2)
                nc.gpsimd.partition_broadcast(out_ap=rbc, in_ap=r_sb)
                if probes and qc == 0 and h == 0:
                    nc.sync.dma_start(out=prb["p_r"], in_=r_sb)
                    nc.sync.dma_start(out=prb["p_rbc"], in_=rbc)
                nc.vector.tensor_mul(
                    CT[pr][HD * off:HD * (off + 1), QCW * qc:QCW * (qc + 1)],
                    ctx_ps[0:HD, :], rbc)
            for mq in range(QCW // P):
                out_sb = outp.tile([P, D], F32, name="out_sb")
                q0 = QCW * qc + P * mq
                for ne in range(2):
                    o_ps = opool.tile([P, QCW], F32, name="o_ps")
                    for pr2 in range(2):
                        nc.tensor.matmul(
                            o_ps,
                            lhsT=CT[pr2][:, q0:q0 + P],
                            rhs=ow_sb[pr2][:, QCW * ne:QCW * (ne + 1)],
                            start=(pr2 == 0), stop=(pr2 == 1))
                    nc.vector.tensor_copy(out_sb[:, QCW * ne:QCW * (ne + 1)], o_ps)
                nc.sync.dma_start(out=out[q0:q0 + P, :], in_=out_sb)
        if probes:
            nc.sync.dma_start(out=prb["p_ct"], in_=CT[0].bitcast(F32))

    if compile_:
        nc.compile()
    return nc


def _get_nc(mode, has_bias):
    key = (mode, has_bias)
    if key not in _NC_CACHE:
        _NC_CACHE[key] = build_nc(mode, has_bias=has_bias)
    return _NC_CACHE[key]


def _tri_bias():
    g = np.arange(P, dtype=np.int64)
    return np.where(g[None, :] < g[:, None], np.float32(NEG), np.float32(0.0))


def host_prep(query, key, value, attn_mask, q_w, q_b, k_w, k_b, v_w, v_b, o_w, o_b):
    """Build (mode, in_maps) for the 8 cores."""
    mask = np.asarray(attn_mask).astype(bool)
    if np.array_equal(mask, np.triu(np.ones((S, S), bool), 1)):
        mode = "causal"
    elif not mask.any():
        mode = "nomask"
    else:
        mode = "generic"

    import ml_dtypes
    bf16 = ml_dtypes.bfloat16
    ones_row = np.ones((1, S), bf16)

    def prep_x(x):
        return np.vstack([np.ascontiguousarray(x.T).astype(bf16), ones_row])

    xs = {}
    for b in range(2):
        xs[b] = (prep_x(np.asarray(query)[b]), prep_x(np.asarray(key)[b]),
                 prep_x(np.asarray(value)[b]))

    tri = _tri_bias()
    biasT = None
    if mode == "generic":
        biasT = np.ascontiguousarray(
            np.where(mask, np.float32(NEG), np.float32(0.0)).T)

    def prep_w(w, bvec, sl):
        return np.vstack([
            np.ascontiguousarray(np.asarray(w)[sl].T).astype(bf16),
            np.asarray(bvec)[sl][None, :].astype(bf16)])

    in_maps = []
    for c in range(8):
        b, g = divmod(c, 4)
        sl = slice(M_DIM * g, M_DIM * (g + 1))
        m = {
            "xqT": xs[b][0], "xkT": xs[b][1], "xvT": xs[b][2],
            "wqT": prep_w(q_w, q_b, sl),
            "wkT": prep_w(k_w, k_b, sl),
            "wvT": prep_w(v_w, v_b, sl),
            "owT": np.ascontiguousarray(np.asarray(o_w)[:, sl].T, dtype=np.float32),
            "btri": tri,
        }
        if mode == "generic":
            m["biasT"] = biasT
        in_maps.append(m)
    return mode, in_maps


def kernel(**inputs) -> np.ndarray:
    global LAST_RESULTS
    from concourse.bass_utils import run_bass_kernel_spmd

    mode, in_maps = host_prep(**inputs)
    has_bias = any(
        np.asarray(inputs[k]).any() for k in ("q_b", "k_b", "v_b"))
    nc = _get_nc(mode, has_bias)
    res = run_bass_kernel_spmd(nc, in_maps, core_ids=list(range(8)), trace=TRACE)
    LAST_RESULTS = res
    parts = [res.results[c]["out"] for c in range(8)]
    o_b = np.asarray(inputs["o_b"]).astype(np.float32)
    out = np.stack([
        parts[0] + parts[1] + parts[2] + parts[3],
        parts[4] + parts[5] + parts[6] + parts[7],
    ], axis=0) + o_b[None, None, :]
    return out.astype(np.float32)


# revision 42
# speedup vs baseline: 1.4328x; 1.3234x over previous
"""Multi-head attention (B=2, S=2048, D=1024, H=16, causal mask) on 8 TRN2 cores.

Sharding: core c handles batch b = c//4 and 4 heads g = c%4 (dims 256g..256g+256
of the projection space).  Each core computes a partial output [S, D] (its 4
heads' contribution to the out-projection); the host sums the 4 partials per
batch and adds the output bias.

Device layout (per core) keeps the sequence axis on the SBUF free dimension:
  QT, KT  [256, 2048]  (head-dim on partitions, 2 head-pairs of 128)
  V_aug   16 tiles [128, 4, 65]  (seq on partitions; per head 64 dims + ones col)
  scores  S.T tiles [128 k, 512 q] per head; causal blocks above diagonal skipped
  exp     ScalarE, scale=1/8, mask folded in as a -1e9 bias (one [128,128] tri tile)
  ctx.T   [65, 512] PSUM per (head, q-chunk); row 64 = softmax denominator l
  norm    reciprocal_approx_fast on l, partition_broadcast, DVE multiply
  out     ctxT (4 heads stacked, [256, 2048]) @ o_w slice -> [2048, 1024]
All matmuls run as float32r (f32 storage bitcast; full PE rate at N>=256).
"""

import numpy as np
from contextlib import ExitStack

import concourse.bacc as bacc
import concourse.bass as bass
import concourse.tile as tile
from concourse import mybir

P = 128
S = 2048
D = 1024
N_HEADS_TOT = 16
HEADS = 4            # per core
HD = 64
M_DIM = HEADS * HD   # 256
KC = 8               # embed-dim 128-chunks
QCW = 512            # q chunk width
NQC = S // QCW       # 4
NKT = S // P         # 16 k-tiles
F32 = mybir.dt.float32
F32R = mybir.dt.float32r
BF16 = mybir.dt.bfloat16
EXPF = mybir.ActivationFunctionType.Exp
NEG = -1.0e9

TRACE = False
LAST_RESULTS = None
_NC_CACHE = {}


def build_nc(mode: str, compile_: bool = True, probes: bool = False,
             has_bias: bool = False) -> bass.Bass:
    """mode in {causal, nomask, generic}"""
    nc = bacc.Bacc("TRN2", target_bir_lowering=False, debug=False)
    prb = {}
    if probes:
        for nm, shape in (("p_qt", [P, S]), ("p_kt", [P, S]), ("p_va", [P, HEADS * (HD + 1)]),
                          ("p_s", [P, QCW]), ("p_p", [P, QCW]), ("p_ctx", [HD + 1, QCW]),
                          ("p_r", [1, QCW]), ("p_rbc", [HD, QCW]), ("p_ct", [P, S])):
            prb[nm] = nc.dram_tensor(nm, shape, F32, kind="ExternalOutput").ap()
    xq = nc.dram_tensor("xqT", [D + 1, S], BF16, kind="ExternalInput").ap()
    xk = nc.dram_tensor("xkT", [D + 1, S], BF16, kind="ExternalInput").ap()
    xv = nc.dram_tensor("xvT", [D + 1, S], BF16, kind="ExternalInput").ap()
    wq = nc.dram_tensor("wqT", [D + 1, M_DIM], BF16, kind="ExternalInput").ap()
    wk = nc.dram_tensor("wkT", [D + 1, M_DIM], BF16, kind="ExternalInput").ap()
    wv = nc.dram_tensor("wvT", [D + 1, M_DIM], BF16, kind="ExternalInput").ap()
    ow = nc.dram_tensor("owT", [M_DIM, D], BF16, kind="ExternalInput").ap()
    btri = nc.dram_tensor("btri", [P, P], F32, kind="ExternalInput").ap()
    bfull = None
    if mode == "generic":
        bfull = nc.dram_tensor("biasT", [S, S], F32, kind="ExternalInput").ap()
    out = nc.dram_tensor("out", [S, D], F32, kind="ExternalOutput").ap()

    with tile.TileContext(nc) as tc, ExitStack() as ctx:
        consts = ctx.enter_context(tc.tile_pool(name="consts", bufs=1))
        xpool = ctx.enter_context(tc.tile_pool(name="xpool", bufs=8))
        qkv = ctx.enter_context(tc.tile_pool(name="qkv", bufs=1))
        ppool = ctx.enter_context(tc.tile_pool(name="ppool", bufs=4))
        bpool = ctx.enter_context(tc.tile_pool(name="bpool", bufs=2))
        small = ctx.enter_context(tc.tile_pool(name="small", bufs=4))
        outp = ctx.enter_context(tc.tile_pool(name="outp", bufs=1 if probes else 2))
        spool = ctx.enter_context(tc.tile_pool(name="spsum", bufs=4, space="PSUM"))
        cpool = ctx.enter_context(tc.tile_pool(name="cpsum", bufs=2, space="PSUM"))
        opool = ctx.enter_context(tc.tile_pool(name="opsum", bufs=2, space="PSUM"))

        # ---- resident weights ----
        def load_w(ap_dram, nm):
            tiles = []
            for kc in range(KC):
                t = consts.tile([P, M_DIM], BF16, name=f"{nm}{kc}")
                nc.sync.dma_start(out=t, in_=ap_dram[P * kc:P * (kc + 1), :])
                tiles.append(t)
            aug = None
            if has_bias:
                aug = consts.tile([1, M_DIM], BF16, name=f"{nm}_aug")
                nc.sync.dma_start(out=aug, in_=ap_dram[D:D + 1, :])
            return tiles, aug

        wq_sb, wq_aug = load_w(wq, "wq")
        wk_sb, wk_aug = load_w(wk, "wk")
        wv_sb, wv_aug = load_w(wv, "wv")
        ow_sb = []
        for pr in range(2):
            t = consts.tile([P, D], BF16, name=f"ow{pr}")
            nc.sync.dma_start(out=t, in_=ow[P * pr:P * (pr + 1), :])
            ow_sb.append(t)
        btri_sb = consts.tile([P, P], F32, name="btri_sb")
        nc.sync.dma_start(out=btri_sb, in_=btri)
        ones4 = consts.tile([P, HEADS], F32, name="ones4")
        nc.vector.memset(ones4, 1.0)

        QT = [qkv.tile([P, S], BF16, name=f"QT{pr}") for pr in range(2)]
        KT = [qkv.tile([P, S], BF16, name=f"KT{pr}") for pr in range(2)]
        CT = [qkv.tile([P, S], BF16, name=f"CT{pr}") for pr in range(2)]
        VA = [qkv.tile([P, HEADS, HD + 1], BF16, name=f"VA{t}") for t in range(NKT)]

        # ---- Q/K projections:  dest[pair][:, n] = w.T @ xT  ----
        for xap, w_sb, w_aug, dest in ((xq, wq_sb, wq_aug, QT), (xk, wk_sb, wk_aug, KT)):
            x_sb = []
            for kc in range(KC):
                xt = xpool.tile([P, S], BF16, name="xt")
                nc.sync.dma_start(out=xt, in_=xap[P * kc:P * (kc + 1), :])
                x_sb.append(xt)
            xaug = None
            if has_bias:
                xaug = small.tile([1, S], BF16, name="xaug", bufs=2)
                nc.sync.dma_start(out=xaug, in_=xap[D:D + 1, :])
            for m in range(2):
                for n in range(NQC):
                    ps = spool.tile([P, QCW], F32, name="s_ps")
                    for kc in range(KC):
                        nc.tensor.matmul(
                            ps,
                            lhsT=w_sb[kc][:, P * m:P * (m + 1)],
                            rhs=x_sb[kc][:, QCW * n:QCW * (n + 1)],
                            start=(kc == 0),
                            stop=(not has_bias and kc == KC - 1))
                    if has_bias:
                        nc.tensor.matmul(
                            ps,
                            lhsT=w_aug[0:1, P * m:P * (m + 1)],
                            rhs=xaug[0:1, QCW * n:QCW * (n + 1)],
                            start=False, stop=True)
                    nc.vector.tensor_copy(dest[m][:, QCW * n:QCW * (n + 1)], ps)

        # ---- V projection (natural layout + ones column) ----
        xv_sb = []
        for kc in range(KC):
            xt = xpool.tile([P, S], BF16, name="xt")
            nc.sync.dma_start(out=xt, in_=xv[P * kc:P * (kc + 1), :])
            xv_sb.append(xt)
        xv_aug = None
        if has_bias:
            xv_aug = small.tile([1, S], BF16, name="xaug", bufs=2)
            nc.sync.dma_start(out=xv_aug, in_=xv[D:D + 1, :])
        for m in range(NKT):
            ps = spool.tile([P, QCW], F32, name="s_ps")
            for kc in range(KC):
                nc.tensor.matmul(
                    ps[:, 0:M_DIM],
                    lhsT=xv_sb[kc][:, P * m:P * (m + 1)],
                    rhs=wv_sb[kc],
                    start=(kc == 0),
                    stop=(not has_bias and kc == KC - 1))
            if has_bias:
                nc.tensor.matmul(
                    ps[:, 0:M_DIM],
                    lhsT=xv_aug[0:1, P * m:P * (m + 1)],
                    rhs=wv_aug,
                    start=False, stop=True)
            nc.vector.tensor_copy(
                VA[m][:, :, 0:HD],
                ps[:, 0:M_DIM].rearrange("p (h d) -> p h d", h=HEADS))
            nc.vector.tensor_copy(
                VA[m][:, :, HD:HD + 1],
                ones4.rearrange("p (h o) -> p h o", o=1))

        if probes:
            nc.sync.dma_start(out=prb["p_qt"].bitcast(BF16)[:, 0:S], in_=QT[0])
            nc.sync.dma_start(out=prb["p_kt"].bitcast(BF16)[:, 0:S], in_=KT[0])
            nc.sync.dma_start(
                out=prb["p_va"].bitcast(BF16)[:, 0:HEADS * (HD + 1)],
                in_=VA[0].rearrange("p h d -> p (h d)"))

        # ---- attention + out-projection ----
        for qc in range(NQC):
            for h in range(HEADS):
                pr, off = divmod(h, 2)
                ctx_ps = cpool.tile([HD + 1, QCW], F32, name="ctx_ps")
                nt = 4 * qc + 4 if mode == "causal" else NKT
                queue = []

                def flush_ctx():
                    t0, p0, o0 = queue.pop(0)
                    nc.tensor.matmul(
                        ctx_ps[:, o0:],
                        lhsT=VA[t0][:, h, :],
                        rhs=p0[:, o0:],
                        start=(t0 == 0), stop=(t0 == nt - 1),
                        skip_group_check=True)

                for t in range(nt):
                    o = max(0, P * t - QCW * qc) if mode == "causal" else 0
                    s_ps = spool.tile([P, QCW], F32, name="s_ps")
                    nc.tensor.matmul(
                        s_ps[:, o:],
                        lhsT=KT[pr][HD * off:HD * (off + 1), P * t:P * (t + 1)],
                        rhs=QT[pr][HD * off:HD * (off + 1), QCW * qc + o:QCW * (qc + 1)],
                        start=True, stop=True)
                    if mode == "causal" and t >= 4 * qc:
                        nc.vector.tensor_add(
                            s_ps[:, o:o + P], s_ps[:, o:o + P], btri_sb)
                    elif mode == "generic":
                        bt = bpool.tile([P, QCW], F32, name="bt")
                        nc.sync.dma_start(
                            out=bt,
                            in_=bfull[P * t:P * (t + 1), QCW * qc:QCW * (qc + 1)])
                        nc.vector.tensor_add(s_ps, s_ps, bt)
                    if probes and qc == 0 and h == 0 and t == 0:
                        dbg = ppool.tile([P, QCW], F32, name="dbg", bufs=1)
                        nc.vector.tensor_copy(dbg, s_ps)
                        nc.sync.dma_start(out=prb["p_s"], in_=dbg)
                    p_sb = ppool.tile([P, QCW], BF16, name="p_sb")
                    nc.scalar.activation(p_sb[:, o:], s_ps[:, o:], EXPF, scale=0.125)
                    if probes and qc == 0 and h == 0 and t == 0:
                        nc.sync.dma_start(out=prb["p_p"].bitcast(BF16)[:, 0:QCW], in_=p_sb)
                    queue.append((t, p_sb, o))
                    if len(queue) > 2:
                        flush_ctx()
                while queue:
                    flush_ctx()
                if probes and qc == 0 and h == 0:
                    dbg2 = ppool.tile([HD + 1, QCW], F32, name="dbg", bufs=1)
                    nc.vector.tensor_copy(dbg2, ctx_ps)
                    nc.sync.dma_start(out=prb["p_ctx"], in_=dbg2)
                l_sb = small.tile([1, QCW], F32, name="l_sb", bufs=3)
                nc.vector.tensor_copy(l_sb, ctx_ps[HD:HD + 1, :])
                r_sb = small.tile([1, QCW], F32, name="r_sb", bufs=3)
                nc.vector.reciprocal_approx_fast(out=r_sb, in_=l_sb)
                rbc = ppool.tile([HD, QCW], F32, name="rbc", bufs=2)
                nc.gpsimd.partition_broadcast(out_ap=rbc, in_ap=r_sb)
                if probes and qc == 0 and h == 0:
                    nc.sync.dma_start(out=prb["p_r"], in_=r_sb)
                    nc.sync.dma_start(out=prb["p_rbc"], in_=rbc)
                nc.vector.tensor_mul(
                    CT[pr][HD * off:HD * (off + 1), QCW * qc:QCW * (qc + 1)],
                    ctx_ps[0:HD, :], rbc)
            for mq in range(QCW // P):
                out_sb = outp.tile([P, D], F32, name="out_sb")
                q0 = QCW * qc + P * mq
                for ne in range(2):
                    o_ps = opool.tile([P, QCW], F32, name="o_ps")
                    for pr2 in range(2):
                        nc.tensor.matmul(
                            o_ps,
                            lhsT=CT[pr2][:, q0:q0 + P],
                            rhs=ow_sb[pr2][:, QCW * ne:QCW * (ne + 1)],
                            start=(pr2 == 0), stop=(pr2 == 1))
                    nc.vector.tensor_copy(out_sb[:, QCW * ne:QCW * (ne + 1)], o_ps)
                nc.sync.dma_start(out=out[q0:q0 + P, :], in_=out_sb)
        if probes:
            nc.sync.dma_start(out=prb["p_ct"].bitcast(BF16)[:, 0:S], in_=CT[0])

    if compile_:
        nc.compile()
    return nc


def _get_nc(mode, has_bias):
    key = (mode, has_bias)
    if key not in _NC_CACHE:
        _NC_CACHE[key] = build_nc(mode, has_bias=has_bias)
    return _NC_CACHE[key]


def _tri_bias():
    g = np.arange(P, dtype=np.int64)
    return np.where(g[None, :] < g[:, None], np.float32(NEG), np.float32(0.0))


def host_prep(query, key, value, attn_mask, q_w, q_b, k_w, k_b, v_w, v_b, o_w, o_b):
    """Build (mode, in_maps) for the 8 cores."""
    mask = np.asarray(attn_mask).astype(bool)
    if np.array_equal(mask, np.triu(np.ones((S, S), bool), 1)):
        mode = "causal"
    elif not mask.any():
        mode = "nomask"
    else:
        mode = "generic"

    import ml_dtypes
    bf16 = ml_dtypes.bfloat16
    ones_row = np.ones((1, S), bf16)

    def prep_x(x):
        return np.vstack([np.ascontiguousarray(x.T).astype(bf16), ones_row])

    xs = {}
    for b in range(2):
        xs[b] = (prep_x(np.asarray(query)[b]), prep_x(np.asarray(key)[b]),
                 prep_x(np.asarray(value)[b]))

    tri = _tri_bias()
    biasT = None
    if mode == "generic":
        biasT = np.ascontiguousarray(
            np.where(mask, np.float32(NEG), np.float32(0.0)).T)

    def prep_w(w, bvec, sl):
        return np.vstack([
            np.ascontiguousarray(np.asarray(w)[sl].T).astype(bf16),
            np.asarray(bvec)[sl][None, :].astype(bf16)])

    in_maps = []
    for c in range(8):
        b, g = divmod(c, 4)
        sl = slice(M_DIM * g, M_DIM * (g + 1))
        m = {
            "xqT": xs[b][0], "xkT": xs[b][1], "xvT": xs[b][2],
            "wqT": prep_w(q_w, q_b, sl),
            "wkT": prep_w(k_w, k_b, sl),
            "wvT": prep_w(v_w, v_b, sl),
            "owT": np.ascontiguousarray(np.asarray(o_w)[:, sl].T).astype(bf16),
            "btri": tri,
        }
        if mode == "generic":
            m["biasT"] = biasT
        in_maps.append(m)
    return mode, in_maps


def kernel(**inputs) -> np.ndarray:
    global LAST_RESULTS
    from concourse.bass_utils import run_bass_kernel_spmd

    mode, in_maps = host_prep(**inputs)
    has_bias = any(
        np.asarray(inputs[k]).any() for k in ("q_b", "k_b", "v_b"))
    nc = _get_nc(mode, has_bias)
    res = run_bass_kernel_spmd(nc, in_maps, core_ids=list(range(8)), trace=TRACE)
    LAST_RESULTS = res
    parts = [res.results[c]["out"] for c in range(8)]
    o_b = np.asarray(inputs["o_b"]).astype(np.float32)
    out = np.stack([
        parts[0] + parts[1] + parts[2] + parts[3],
        parts[4] + parts[5] + parts[6] + parts[7],
    ], axis=0) + o_b[None, None, :]
    return out.astype(np.float32)


# revision 43
# speedup vs baseline: 1.5986x; 1.1158x over previous
"""Multi-head attention (B=2, S=2048, D=1024, H=16, causal mask) on 8 TRN2 cores.

Sharding: core c handles batch b = c//4 and 4 heads g = c%4 (dims 256g..256g+256
of the projection space).  Each core computes a partial output [S, D] (its 4
heads' contribution to the out-projection); the host sums the 4 partials per
batch and adds the output bias.

Device layout (per core) keeps the sequence axis on the SBUF free dimension:
  QT, KT  [256, 2048]  (head-dim on partitions, 2 head-pairs of 128)
  V_aug   16 tiles [128, 4, 65]  (seq on partitions; per head 64 dims + ones col)
  scores  S.T tiles [128 k, 512 q] per head; causal blocks above diagonal skipped
  exp     ScalarE, scale=1/8, mask folded in as a -1e9 bias (one [128,128] tri tile)
  ctx.T   [65, 512] PSUM per (head, q-chunk); row 64 = softmax denominator l
  norm    reciprocal_approx_fast on l, partition_broadcast, DVE multiply
  out     ctxT (4 heads stacked, [256, 2048]) @ o_w slice -> [2048, 1024]
All matmuls run as float32r (f32 storage bitcast; full PE rate at N>=256).
"""

import numpy as np
from contextlib import ExitStack

import concourse.bacc as bacc
import concourse.bass as bass
import concourse.tile as tile
from concourse import mybir

P = 128
S = 2048
D = 1024
N_HEADS_TOT = 16
HEADS = 4            # per core
HD = 64
M_DIM = HEADS * HD   # 256
KC = 8               # embed-dim 128-chunks
QCW = 512            # q chunk width
NQC = S // QCW       # 4
NKT = S // P         # 16 k-tiles
F32 = mybir.dt.float32
F32R = mybir.dt.float32r
BF16 = mybir.dt.bfloat16
EXPF = mybir.ActivationFunctionType.Exp
NEG = -1.0e9

TRACE = False
LAST_RESULTS = None
_NC_CACHE = {}


def build_nc(mode: str, compile_: bool = True, probes: bool = False,
             has_bias: bool = False) -> bass.Bass:
    """mode in {causal, nomask, generic}"""
    nc = bacc.Bacc("TRN2", target_bir_lowering=False, debug=False)
    prb = {}
    if probes:
        for nm, shape in (("p_qt", [P, S]), ("p_kt", [P, S]), ("p_va", [P, HEADS * (HD + 1)]),
                          ("p_s", [P, QCW]), ("p_p", [P, QCW]), ("p_ctx", [HD + 1, QCW]),
                          ("p_r", [1, QCW]), ("p_rbc", [HD, QCW]), ("p_ct", [P, S])):
            prb[nm] = nc.dram_tensor(nm, shape, F32, kind="ExternalOutput").ap()
    xq = nc.dram_tensor("xqT", [D + 1, S], BF16, kind="ExternalInput").ap()
    xk = nc.dram_tensor("xkT", [D + 1, S], BF16, kind="ExternalInput").ap()
    xv = nc.dram_tensor("xvT", [D + 1, S], BF16, kind="ExternalInput").ap()
    wq = nc.dram_tensor("wqT", [D + 1, M_DIM], BF16, kind="ExternalInput").ap()
    wk = nc.dram_tensor("wkT", [D + 1, M_DIM], BF16, kind="ExternalInput").ap()
    wv = nc.dram_tensor("wvT", [D + 1, M_DIM], BF16, kind="ExternalInput").ap()
    ow = nc.dram_tensor("owT", [M_DIM, D], BF16, kind="ExternalInput").ap()
    btri = nc.dram_tensor("btri", [P, P], F32, kind="ExternalInput").ap()
    bfull = None
    if mode == "generic":
        bfull = nc.dram_tensor("biasT", [S, S], F32, kind="ExternalInput").ap()
    out = nc.dram_tensor("out", [S, D], F32, kind="ExternalOutput").ap()

    with tile.TileContext(nc) as tc, ExitStack() as ctx:
        consts = ctx.enter_context(tc.tile_pool(name="consts", bufs=1))
        xpool = ctx.enter_context(tc.tile_pool(name="xpool", bufs=16))
        qkv = ctx.enter_context(tc.tile_pool(name="qkv", bufs=1))
        ppool = ctx.enter_context(tc.tile_pool(name="ppool", bufs=6))
        bpool = ctx.enter_context(tc.tile_pool(name="bpool", bufs=2))
        small = ctx.enter_context(tc.tile_pool(name="small", bufs=4))
        outp = ctx.enter_context(tc.tile_pool(name="outp", bufs=1 if probes else 2))
        spool = ctx.enter_context(tc.tile_pool(name="spsum", bufs=4, space="PSUM"))
        cpool = ctx.enter_context(tc.tile_pool(name="cpsum", bufs=2, space="PSUM"))
        opool = ctx.enter_context(tc.tile_pool(name="opsum", bufs=2, space="PSUM"))

        # ---- resident weights ----
        def load_w(ap_dram, nm):
            tiles = []
            for kc in range(KC):
                t = consts.tile([P, M_DIM], BF16, name=f"{nm}{kc}")
                nc.sync.dma_start(out=t, in_=ap_dram[P * kc:P * (kc + 1), :])
                tiles.append(t)
            aug = None
            if has_bias:
                aug = consts.tile([1, M_DIM], BF16, name=f"{nm}_aug")
                nc.sync.dma_start(out=aug, in_=ap_dram[D:D + 1, :])
            return tiles, aug

        wq_sb, wq_aug = load_w(wq, "wq")
        wk_sb, wk_aug = load_w(wk, "wk")
        wv_sb, wv_aug = load_w(wv, "wv")
        ow_sb = []
        for pr in range(2):
            t = consts.tile([P, D], BF16, name=f"ow{pr}")
            nc.sync.dma_start(out=t, in_=ow[P * pr:P * (pr + 1), :])
            ow_sb.append(t)
        btri_sb = consts.tile([P, P], F32, name="btri_sb")
        nc.sync.dma_start(out=btri_sb, in_=btri)
        ones4 = consts.tile([P, HEADS], F32, name="ones4")
        nc.vector.memset(ones4, 1.0)

        QT = [qkv.tile([P, S], BF16, name=f"QT{pr}") for pr in range(2)]
        KT = [qkv.tile([P, S], BF16, name=f"KT{pr}") for pr in range(2)]
        CT = [qkv.tile([P, S], BF16, name=f"CT{pr}") for pr in range(2)]
        VA = [qkv.tile([P, HEADS, HD + 1], BF16, name=f"VA{t}") for t in range(NKT)]

        # ---- Q/K projections:  dest[pair][:, n] = w.T @ xT  ----
        for xap, w_sb, w_aug, dest in ((xq, wq_sb, wq_aug, QT), (xk, wk_sb, wk_aug, KT)):
            x_sb = []
            for kc in range(KC):
                xt = xpool.tile([P, S], BF16, name="xt")
                nc.sync.dma_start(out=xt, in_=xap[P * kc:P * (kc + 1), :])
                x_sb.append(xt)
            xaug = None
            if has_bias:
                xaug = small.tile([1, S], BF16, name="xaug", bufs=2)
                nc.sync.dma_start(out=xaug, in_=xap[D:D + 1, :])
            for m in range(2):
                for n in range(NQC):
                    ps = spool.tile([P, QCW], F32, name="s_ps")
                    for kc in range(KC):
                        nc.tensor.matmul(
                            ps,
                            lhsT=w_sb[kc][:, P * m:P * (m + 1)],
                            rhs=x_sb[kc][:, QCW * n:QCW * (n + 1)],
                            start=(kc == 0),
                            stop=(not has_bias and kc == KC - 1))
                    if has_bias:
                        nc.tensor.matmul(
                            ps,
                            lhsT=w_aug[0:1, P * m:P * (m + 1)],
                            rhs=xaug[0:1, QCW * n:QCW * (n + 1)],
                            start=False, stop=True)
                    nc.vector.tensor_copy(dest[m][:, QCW * n:QCW * (n + 1)], ps)

        # ---- V projection (natural layout + ones column) ----
        xv_sb = []
        for kc in range(KC):
            xt = xpool.tile([P, S], BF16, name="xt")
            nc.sync.dma_start(out=xt, in_=xv[P * kc:P * (kc + 1), :])
            xv_sb.append(xt)
        xv_aug = None
        if has_bias:
            xv_aug = small.tile([1, S], BF16, name="xaug", bufs=2)
            nc.sync.dma_start(out=xv_aug, in_=xv[D:D + 1, :])
        for m in range(NKT):
            ps = spool.tile([P, QCW], F32, name="s_ps")
            for kc in range(KC):
                nc.tensor.matmul(
                    ps[:, 0:M_DIM],
                    lhsT=xv_sb[kc][:, P * m:P * (m + 1)],
                    rhs=wv_sb[kc],
                    start=(kc == 0),
                    stop=(not has_bias and kc == KC - 1))
            if has_bias:
                nc.tensor.matmul(
                    ps[:, 0:M_DIM],
                    lhsT=xv_aug[0:1, P * m:P * (m + 1)],
                    rhs=wv_aug,
                    start=False, stop=True)
            nc.vector.tensor_copy(
                VA[m][:, :, 0:HD],
                ps[:, 0:M_DIM].rearrange("p (h d) -> p h d", h=HEADS))
            nc.vector.tensor_copy(
                VA[m][:, :, HD:HD + 1],
                ones4.rearrange("p (h o) -> p h o", o=1))

        if probes:
            nc.sync.dma_start(out=prb["p_qt"].bitcast(BF16)[:, 0:S], in_=QT[0])
            nc.sync.dma_start(out=prb["p_kt"].bitcast(BF16)[:, 0:S], in_=KT[0])
            nc.sync.dma_start(
                out=prb["p_va"].bitcast(BF16)[:, 0:HEADS * (HD + 1)],
                in_=VA[0].rearrange("p h d -> p (h d)"))

        # ---- attention + out-projection ----
        for qc in range(NQC):
            for pr in range(2):
                nt = 4 * qc + 4 if mode == "causal" else NKT
                ctxs = [cpool.tile([HD + 1, QCW], F32, name="ctx_ps")
                        for _ in range(2)]
                queues = ([], [])

                def flush_ctx(j):
                    t0, p0, o0 = queues[j].pop(0)
                    nc.tensor.matmul(
                        ctxs[j][:, o0:],
                        lhsT=VA[t0][:, 2 * pr + j, :],
                        rhs=p0[:, o0:],
                        start=(t0 == 0), stop=(t0 == nt - 1),
                        skip_group_check=True)

                for t in range(nt):
                    o = max(0, P * t - QCW * qc) if mode == "causal" else 0
                    for j in range(2):
                        s_ps = spool.tile([P, QCW], F32, name="s_ps")
                        nc.tensor.matmul(
                            s_ps[:, o:],
                            lhsT=KT[pr][HD * j:HD * (j + 1), P * t:P * (t + 1)],
                            rhs=QT[pr][HD * j:HD * (j + 1), QCW * qc + o:QCW * (qc + 1)],
                            start=True, stop=True,
                            tile_position=(HD * j, 0))
                        if mode == "causal" and t >= 4 * qc:
                            nc.vector.tensor_add(
                                s_ps[:, o:o + P], s_ps[:, o:o + P], btri_sb)
                        elif mode == "generic":
                            bt = bpool.tile([P, QCW], F32, name="bt")
                            nc.sync.dma_start(
                                out=bt,
                                in_=bfull[P * t:P * (t + 1), QCW * qc:QCW * (qc + 1)])
                            nc.vector.tensor_add(s_ps, s_ps, bt)
                        if probes and qc == 0 and pr == 0 and j == 0 and t == 0:
                            dbg = ppool.tile([P, QCW], F32, name="dbg", bufs=1)
                            nc.vector.tensor_copy(dbg, s_ps)
                            nc.sync.dma_start(out=prb["p_s"], in_=dbg)
                        p_sb = ppool.tile([P, QCW], BF16, name="p_sb")
                        nc.scalar.activation(
                            p_sb[:, o:], s_ps[:, o:], EXPF, scale=0.125)
                        if probes and qc == 0 and pr == 0 and j == 0 and t == 0:
                            nc.sync.dma_start(
                                out=prb["p_p"].bitcast(BF16)[:, 0:QCW], in_=p_sb)
                        queues[j].append((t, p_sb, o))
                    for j in range(2):
                        if len(queues[j]) > 1:
                            flush_ctx(j)
                for j in range(2):
                    while queues[j]:
                        flush_ctx(j)
                for j in range(2):
                    ctx_ps = ctxs[j]
                    if probes and qc == 0 and pr == 0 and j == 0:
                        dbg2 = ppool.tile([HD + 1, QCW], F32, name="dbg", bufs=1)
                        nc.vector.tensor_copy(dbg2, ctx_ps)
                        nc.sync.dma_start(out=prb["p_ctx"], in_=dbg2)
                    l_sb = small.tile([1, QCW], F32, name="l_sb", bufs=3)
                    nc.vector.tensor_copy(l_sb, ctx_ps[HD:HD + 1, :])
                    r_sb = small.tile([1, QCW], F32, name="r_sb", bufs=3)
                    nc.vector.reciprocal_approx_fast(out=r_sb, in_=l_sb)
                    rbc = ppool.tile([HD, QCW], F32, name="rbc", bufs=2)
                    nc.gpsimd.partition_broadcast(out_ap=rbc, in_ap=r_sb)
                    if probes and qc == 0 and pr == 0 and j == 0:
                        nc.sync.dma_start(out=prb["p_r"], in_=r_sb)
                        nc.sync.dma_start(out=prb["p_rbc"], in_=rbc)
                    nc.vector.tensor_mul(
                        CT[pr][HD * j:HD * (j + 1), QCW * qc:QCW * (qc + 1)],
                        ctx_ps[0:HD, :], rbc)
            for mq in range(QCW // P):
                out_sb = outp.tile([P, D], F32, name="out_sb")
                q0 = QCW * qc + P * mq
                for ne in range(2):
                    o_ps = opool.tile([P, QCW], F32, name="o_ps")
                    for pr2 in range(2):
                        nc.tensor.matmul(
                            o_ps,
                            lhsT=CT[pr2][:, q0:q0 + P],
                            rhs=ow_sb[pr2][:, QCW * ne:QCW * (ne + 1)],
                            start=(pr2 == 0), stop=(pr2 == 1))
                    nc.vector.tensor_copy(out_sb[:, QCW * ne:QCW * (ne + 1)], o_ps)
                nc.sync.dma_start(out=out[q0:q0 + P, :], in_=out_sb)
        if probes:
            nc.sync.dma_start(out=prb["p_ct"].bitcast(BF16)[:, 0:S], in_=CT[0])

    if compile_:
        nc.compile()
    return nc


def _get_nc(mode, has_bias):
    key = (mode, has_bias)
    if key not in _NC_CACHE:
        _NC_CACHE[key] = build_nc(mode, has_bias=has_bias)
    return _NC_CACHE[key]


def _tri_bias():
    g = np.arange(P, dtype=np.int64)
    return np.where(g[None, :] < g[:, None], np.float32(NEG), np.float32(0.0))


def host_prep(query, key, value, attn_mask, q_w, q_b, k_w, k_b, v_w, v_b, o_w, o_b):
    """Build (mode, in_maps) for the 8 cores."""
    mask = np.asarray(attn_mask).astype(bool)
    if np.array_equal(mask, np.triu(np.ones((S, S), bool), 1)):
        mode = "causal"
    elif not mask.any():
        mode = "nomask"
    else:
        mode = "generic"

    import ml_dtypes
    bf16 = ml_dtypes.bfloat16
    ones_row = np.ones((1, S), bf16)

    def prep_x(x):
        return np.vstack([np.ascontiguousarray(x.T).astype(bf16), ones_row])

    xs = {}
    for b in range(2):
        xs[b] = (prep_x(np.asarray(query)[b]), prep_x(np.asarray(key)[b]),
                 prep_x(np.asarray(value)[b]))

    tri = _tri_bias()
    biasT = None
    if mode == "generic":
        biasT = np.ascontiguousarray(
            np.where(mask, np.float32(NEG), np.float32(0.0)).T)

    def prep_w(w, bvec, sl):
        return np.vstack([
            np.ascontiguousarray(np.asarray(w)[sl].T).astype(bf16),
            np.asarray(bvec)[sl][None, :].astype(bf16)])

    in_maps = []
    for c in range(8):
        b, g = divmod(c, 4)
        sl = slice(M_DIM * g, M_DIM * (g + 1))
        m = {
            "xqT": xs[b][0], "xkT": xs[b][1], "xvT": xs[b][2],
            "wqT": prep_w(q_w, q_b, sl),
            "wkT": prep_w(k_w, k_b, sl),
            "wvT": prep_w(v_w, v_b, sl),
            "owT": np.ascontiguousarray(np.asarray(o_w)[:, sl].T).astype(bf16),
            "btri": tri,
        }
        if mode == "generic":
            m["biasT"] = biasT
        in_maps.append(m)
    return mode, in_maps


def kernel(**inputs) -> np.ndarray:
    global LAST_RESULTS
    from concourse.bass_utils import run_bass_kernel_spmd

    mode, in_maps = host_prep(**inputs)
    has_bias = any(
        np.asarray(inputs[k]).any() for k in ("q_b", "k_b", "v_b"))
    nc = _get_nc(mode, has_bias)
    res = run_bass_kernel_spmd(nc, in_maps, core_ids=list(range(8)), trace=TRACE)
    LAST_RESULTS = res
    parts = [res.results[c]["out"] for c in range(8)]
    o_b = np.asarray(inputs["o_b"]).astype(np.float32)
    out = np.stack([
        parts[0] + parts[1] + parts[2] + parts[3],
        parts[4] + parts[5] + parts[6] + parts[7],
    ], axis=0) + o_b[None, None, :]
    return out.astype(np.float32)


# revision 45
# speedup vs baseline: 1.5994x; 1.0005x over previous
"""Multi-head attention (B=2, S=2048, D=1024, H=16, causal mask) on 8 TRN2 cores.

Sharding: core c handles batch b = c//4 and 4 heads g = c%4 (dims 256g..256g+256
of the projection space).  Each core computes a partial output [S, D] (its 4
heads' contribution to the out-projection); the host sums the 4 partials per
batch and adds the output bias.

Device layout (per core) keeps the sequence axis on the SBUF free dimension:
  QT, KT  [256, 2048]  (head-dim on partitions, 2 head-pairs of 128)
  V_aug   16 tiles [128, 4, 65]  (seq on partitions; per head 64 dims + ones col)
  scores  S.T tiles [128 k, 512 q] per head; causal blocks above diagonal skipped
  exp     ScalarE, scale=1/8, mask folded in as a -1e9 bias (one [128,128] tri tile)
  ctx.T   [65, 512] PSUM per (head, q-chunk); row 64 = softmax denominator l
  norm    reciprocal_approx_fast on l, partition_broadcast, DVE multiply
  out     ctxT (4 heads stacked, [256, 2048]) @ o_w slice -> [2048, 1024]
All matmuls run as float32r (f32 storage bitcast; full PE rate at N>=256).
"""

import numpy as np
from contextlib import ExitStack

import concourse.bacc as bacc
import concourse.bass as bass
import concourse.tile as tile
from concourse import mybir

P = 128
S = 2048
D = 1024
N_HEADS_TOT = 16
HEADS = 4            # per core
HD = 64
M_DIM = HEADS * HD   # 256
KC = 8               # embed-dim 128-chunks
QCW = 512            # q chunk width
NQC = S // QCW       # 4
NKT = S // P         # 16 k-tiles
F32 = mybir.dt.float32
F32R = mybir.dt.float32r
BF16 = mybir.dt.bfloat16
EXPF = mybir.ActivationFunctionType.Exp
NEG = -1.0e9

TRACE = False
LAST_RESULTS = None
_NC_CACHE = {}


def build_nc(mode: str, compile_: bool = True, probes: bool = False,
             has_bias: bool = False) -> bass.Bass:
    """mode in {causal, nomask, generic}"""
    nc = bacc.Bacc("TRN2", target_bir_lowering=False, debug=False)
    prb = {}
    if probes:
        for nm, shape in (("p_qt", [P, S]), ("p_kt", [P, S]),
                          ("p_va", [P, HEADS * (HD + 1)]), ("p_ct", [P, S])):
            prb[nm] = nc.dram_tensor(nm, shape, F32, kind="ExternalOutput").ap()
    xq = nc.dram_tensor("xqT", [D + 1, S], BF16, kind="ExternalInput").ap()
    xk = nc.dram_tensor("xkT", [D + 1, S], BF16, kind="ExternalInput").ap()
    xv = nc.dram_tensor("xvT", [D + 1, S], BF16, kind="ExternalInput").ap()
    wq = nc.dram_tensor("wqT", [D + 1, M_DIM], BF16, kind="ExternalInput").ap()
    wk = nc.dram_tensor("wkT", [D + 1, M_DIM], BF16, kind="ExternalInput").ap()
    wv = nc.dram_tensor("wvT", [D + 1, M_DIM], BF16, kind="ExternalInput").ap()
    ow = nc.dram_tensor("owT", [M_DIM, D], BF16, kind="ExternalInput").ap()
    btri = nc.dram_tensor("btri", [P, P], F32, kind="ExternalInput").ap()
    bfull = None
    if mode == "generic":
        bfull = nc.dram_tensor("biasT", [S, S], F32, kind="ExternalInput").ap()
    out = nc.dram_tensor("out", [S, D], BF16, kind="ExternalOutput").ap()

    with tile.TileContext(nc) as tc, ExitStack() as ctx:
        consts = ctx.enter_context(tc.tile_pool(name="consts", bufs=1))
        xpool = ctx.enter_context(tc.tile_pool(name="xpool", bufs=48))
        qkv = ctx.enter_context(tc.tile_pool(name="qkv", bufs=1))
        ppool = ctx.enter_context(tc.tile_pool(name="ppool", bufs=4))
        bpool = ctx.enter_context(tc.tile_pool(name="bpool", bufs=2))
        small = ctx.enter_context(tc.tile_pool(name="small", bufs=4))
        outp = ctx.enter_context(tc.tile_pool(name="outp", bufs=2))
        spool = ctx.enter_context(tc.tile_pool(name="spsum", bufs=2, space="PSUM"))
        cpool = ctx.enter_context(tc.tile_pool(name="cpsum", bufs=2, space="PSUM"))
        opool = ctx.enter_context(tc.tile_pool(name="opsum", bufs=2, space="PSUM"))

        # ---- resident weights ----
        def load_w(ap_dram, nm):
            tiles = []
            for kc in range(KC):
                t = consts.tile([P, M_DIM], BF16, name=f"{nm}{kc}")
                nc.sync.dma_start(out=t, in_=ap_dram[P * kc:P * (kc + 1), :])
                tiles.append(t)
            aug = None
            if has_bias:
                aug = consts.tile([1, M_DIM], BF16, name=f"{nm}_aug")
                nc.sync.dma_start(out=aug, in_=ap_dram[D:D + 1, :])
            return tiles, aug

        wq_sb, wq_aug = load_w(wq, "wq")
        wk_sb, wk_aug = load_w(wk, "wk")
        wv_sb, wv_aug = load_w(wv, "wv")
        ow_sb = []
        for pr in range(2):
            t = consts.tile([P, D], BF16, name=f"ow{pr}")
            nc.sync.dma_start(out=t, in_=ow[P * pr:P * (pr + 1), :])
            ow_sb.append(t)
        btri_sb = consts.tile([P, P], F32, name="btri_sb")
        nc.sync.dma_start(out=btri_sb, in_=btri)
        ones4 = consts.tile([P, HEADS], F32, name="ones4")
        nc.vector.memset(ones4, 1.0)

        QT = [qkv.tile([P, S], BF16, name=f"QT{pr}") for pr in range(2)]
        KT = [qkv.tile([P, S], BF16, name=f"KT{pr}") for pr in range(2)]
        CT = [qkv.tile([P, S], BF16, name=f"CT{pr}") for pr in range(2)]
        VA = [qkv.tile([P, HEADS, HD + 1], BF16, name=f"VA{t}") for t in range(NKT)]

        def load_pieces(xap, n):
            """8 [128, 512] pieces of x.T covering q columns of stage n, plus
            the bias ones-row piece."""
            ps = []
            for kc in range(KC):
                xt = xpool.tile([P, QCW], BF16, name="xt")
                nc.sync.dma_start(
                    out=xt,
                    in_=xap[P * kc:P * (kc + 1), QCW * n:QCW * (n + 1)])
                ps.append(xt)
            aug = None
            if has_bias:
                aug = small.tile([1, QCW], BF16, name="xaug", bufs=3)
                nc.sync.dma_start(
                    out=aug, in_=xap[D:D + 1, QCW * n:QCW * (n + 1)])
            return ps, aug

        for n in range(NQC):
            # ---- stage n projections: q/k columns + v rows [512n, 512n+512) ----
            xq_p, xq_a = load_pieces(xq, n)
            xk_p, xk_a = load_pieces(xk, n)
            xv_p, xv_a = load_pieces(xv, n)
            for (x_p, x_a, w_sb, w_aug, dest) in (
                    (xq_p, xq_a, wq_sb, wq_aug, QT),
                    (xk_p, xk_a, wk_sb, wk_aug, KT)):
                for m in range(2):
                    ps = spool.tile([P, 2, QCW], F32, name="s_ps")
                    for kc in range(KC):
                        nc.tensor.matmul(
                            ps[:, 0, :],
                            lhsT=w_sb[kc][:, P * m:P * (m + 1)],
                            rhs=x_p[kc],
                            start=(kc == 0),
                            stop=(not has_bias and kc == KC - 1))
                    if has_bias:
                        nc.tensor.matmul(
                            ps[:, 0, :],
                            lhsT=w_aug[0:1, P * m:P * (m + 1)],
                            rhs=x_a,
                            start=False, stop=True)
                    nc.vector.tensor_copy(
                        dest[m][:, QCW * n:QCW * (n + 1)], ps[:, 0, :])
            for mv in range(4):
                m = 4 * n + mv
                ps = spool.tile([P, 2, QCW], F32, name="s_ps")
                for kc in range(KC):
                    nc.tensor.matmul(
                        ps[:, 0, 0:M_DIM],
                        lhsT=xv_p[kc][:, P * mv:P * (mv + 1)],
                        rhs=wv_sb[kc],
                        start=(kc == 0),
                        stop=(not has_bias and kc == KC - 1))
                if has_bias:
                    nc.tensor.matmul(
                        ps[:, 0, 0:M_DIM],
                        lhsT=xv_a[0:1, P * mv:P * (mv + 1)],
                        rhs=wv_aug,
                        start=False, stop=True)
                nc.vector.tensor_copy(
                    VA[m][:, :, 0:HD],
                    ps[:, 0, 0:M_DIM].rearrange("p (h d) -> p h d", h=HEADS))
                nc.vector.tensor_copy(
                    VA[m][:, :, HD:HD + 1],
                    ones4.rearrange("p (h o) -> p h o", o=1))
            if probes and n == NQC - 1:
                nc.sync.dma_start(out=prb["p_qt"].bitcast(BF16)[:, 0:S], in_=QT[0])
                nc.sync.dma_start(out=prb["p_kt"].bitcast(BF16)[:, 0:S], in_=KT[0])
                nc.sync.dma_start(
                    out=prb["p_va"].bitcast(BF16)[:, 0:HEADS * (HD + 1)],
                    in_=VA[0].rearrange("p h d -> p (h d)"))

            # ---- stage n attention (q chunk n) ----
            qc = n
            for pr in range(2):
                nt = 4 * qc + 4 if mode == "causal" else NKT
                ctxs = [cpool.tile([HD + 1, QCW], F32, name="ctx_ps")
                        for _ in range(2)]
                queues = ([], [])

                def flush_ctx(j):
                    t0, p0, o0 = queues[j].pop(0)
                    nc.tensor.matmul(
                        ctxs[j][:, o0:],
                        lhsT=VA[t0][:, 2 * pr + j, :],
                        rhs=p0[:, j, o0:],
                        start=(t0 == 0), stop=(t0 == nt - 1),
                        skip_group_check=True)

                for t in range(nt):
                    o = max(0, P * t - QCW * qc) if mode == "causal" else 0
                    s_ps = spool.tile([P, 2, QCW], F32, name="s_ps")
                    for j in range(2):
                        nc.tensor.matmul(
                            s_ps[:, j, o:],
                            lhsT=KT[pr][HD * j:HD * (j + 1), P * t:P * (t + 1)],
                            rhs=QT[pr][HD * j:HD * (j + 1),
                                       QCW * qc + o:QCW * (qc + 1)],
                            start=True, stop=True,
                            tile_position=(HD * j, 0))
                    if mode == "causal" and t >= 4 * qc:
                        nc.vector.tensor_add(
                            s_ps[:, :, o:o + P],
                            s_ps[:, :, o:o + P],
                            btri_sb.rearrange("p (a q) -> p a q", a=1)
                            .to_broadcast([P, 2, P]))
                    elif mode == "generic":
                        bt = bpool.tile([P, QCW], F32, name="bt")
                        nc.sync.dma_start(
                            out=bt,
                            in_=bfull[P * t:P * (t + 1), QCW * qc:QCW * (qc + 1)])
                        nc.vector.tensor_add(
                            s_ps, s_ps,
                            bt.rearrange("p (a q) -> p a q", a=1)
                            .to_broadcast([P, 2, QCW]))
                    p_sb = ppool.tile([P, 2, QCW], BF16, name="p_sb")
                    nc.scalar.activation(
                        p_sb[:, :, o:], s_ps[:, :, o:], EXPF, scale=0.125)
                    for j in range(2):
                        queues[j].append((t, p_sb, o))
                    for j in range(2):
                        if len(queues[j]) > 1:
                            flush_ctx(j)
                for j in range(2):
                    while queues[j]:
                        flush_ctx(j)
                for j in range(2):
                    ctx_ps = ctxs[j]
                    l_sb = small.tile([1, QCW], F32, name="l_sb", bufs=3)
                    nc.vector.tensor_copy(l_sb, ctx_ps[HD:HD + 1, :])
                    r_sb = small.tile([1, QCW], F32, name="r_sb", bufs=3)
                    nc.vector.reciprocal_approx_fast(out=r_sb, in_=l_sb)
                    rbc = ppool.tile([HD, QCW], F32, name="rbc", bufs=2)
                    nc.gpsimd.partition_broadcast(out_ap=rbc, in_ap=r_sb)
                    nc.vector.tensor_mul(
                        CT[pr][HD * j:HD * (j + 1), QCW * qc:QCW * (qc + 1)],
                        ctx_ps[0:HD, :], rbc)

            # ---- stage n out-projection ----
            for mq in range(QCW // P):
                out_sb = outp.tile([P, D], BF16, name="out_sb")
                q0 = QCW * qc + P * mq
                for ne in range(2):
                    o_ps = opool.tile([P, QCW], F32, name="o_ps")
                    for pr2 in range(2):
                        nc.tensor.matmul(
                            o_ps,
                            lhsT=CT[pr2][:, q0:q0 + P],
                            rhs=ow_sb[pr2][:, QCW * ne:QCW * (ne + 1)],
                            start=(pr2 == 0), stop=(pr2 == 1))
                    nc.vector.tensor_copy(out_sb[:, QCW * ne:QCW * (ne + 1)], o_ps)
                nc.sync.dma_start(out=out[q0:q0 + P, :], in_=out_sb)
        if probes:
            nc.sync.dma_start(out=prb["p_ct"].bitcast(BF16)[:, 0:S], in_=CT[0])

    if compile_:
        nc.compile()
    return nc


def _get_nc(mode, has_bias):
    key = (mode, has_bias)
    if key not in _NC_CACHE:
        _NC_CACHE[key] = build_nc(mode, has_bias=has_bias)
    return _NC_CACHE[key]


def _tri_bias():
    g = np.arange(P, dtype=np.int64)
    return np.where(g[None, :] < g[:, None], np.float32(NEG), np.float32(0.0))


def host_prep(query, key, value, attn_mask, q_w, q_b, k_w, k_b, v_w, v_b, o_w, o_b):
    """Build (mode, in_maps) for the 8 cores."""
    mask = np.asarray(attn_mask).astype(bool)
    if np.array_equal(mask, np.triu(np.ones((S, S), bool), 1)):
        mode = "causal"
    elif not mask.any():
        mode = "nomask"
    else:
        mode = "generic"

    import ml_dtypes
    bf16 = ml_dtypes.bfloat16
    ones_row = np.ones((1, S), bf16)

    def prep_x(x):
        return np.vstack([np.ascontiguousarray(x.T).astype(bf16), ones_row])

    xs = {}
    for b in range(2):
        xs[b] = (prep_x(np.asarray(query)[b]), prep_x(np.asarray(key)[b]),
                 prep_x(np.asarray(value)[b]))

    tri = _tri_bias()
    biasT = None
    if mode == "generic":
        biasT = np.ascontiguousarray(
            np.where(mask, np.float32(NEG), np.float32(0.0)).T)

    def prep_w(w, bvec, sl):
        return np.vstack([
            np.ascontiguousarray(np.asarray(w)[sl].T).astype(bf16),
            np.asarray(bvec)[sl][None, :].astype(bf16)])

    in_maps = []
    for c in range(8):
        b, g = divmod(c, 4)
        sl = slice(M_DIM * g, M_DIM * (g + 1))
        m = {
            "xqT": xs[b][0], "xkT": xs[b][1], "xvT": xs[b][2],
            "wqT": prep_w(q_w, q_b, sl),
            "wkT": prep_w(k_w, k_b, sl),
            "wvT": prep_w(v_w, v_b, sl),
            "owT": np.ascontiguousarray(np.asarray(o_w)[:, sl].T).astype(bf16),
            "btri": tri,
        }
        if mode == "generic":
            m["biasT"] = biasT
        in_maps.append(m)
    return mode, in_maps


def kernel(**inputs) -> np.ndarray:
    global LAST_RESULTS
    from concourse.bass_utils import run_bass_kernel_spmd

    mode, in_maps = host_prep(**inputs)
    has_bias = any(
        np.asarray(inputs[k]).any() for k in ("q_b", "k_b", "v_b"))
    nc = _get_nc(mode, has_bias)
    res = run_bass_kernel_spmd(nc, in_maps, core_ids=list(range(8)), trace=TRACE)
    LAST_RESULTS = res
    parts = [np.asarray(res.results[c]["out"]).astype(np.float32)
             for c in range(8)]
    o_b = np.asarray(inputs["o_b"]).astype(np.float32)
    out = np.stack([
        parts[0] + parts[1] + parts[2] + parts[3],
        parts[4] + parts[5] + parts[6] + parts[7],
    ], axis=0) + o_b[None, None, :]
    return out.astype(np.float32)


# revision 46
# speedup vs baseline: 1.6479x; 1.0303x over previous
"""Multi-head attention (B=2, S=2048, D=1024, H=16, causal mask) on 8 TRN2 cores.

Sharding: core c handles batch b = c//4 and 4 heads g = c%4 (dims 256g..256g+256
of the projection space).  Each core computes a partial output [S, D] (its 4
heads' contribution to the out-projection); the host sums the 4 partials per
batch and adds the output bias.

Device layout (per core) keeps the sequence axis on the SBUF free dimension:
  QT, KT  [256, 2048]  (head-dim on partitions, 2 head-pairs of 128)
  V_aug   16 tiles [128, 4, 65]  (seq on partitions; per head 64 dims + ones col)
  scores  S.T tiles [128 k, 512 q] per head; causal blocks above diagonal skipped
  exp     ScalarE, scale=1/8, mask folded in as a -1e9 bias (one [128,128] tri tile)
  ctx.T   [65, 512] PSUM per (head, q-chunk); row 64 = softmax denominator l
  norm    reciprocal_approx_fast on l, partition_broadcast, DVE multiply
  out     ctxT (4 heads stacked, [256, 2048]) @ o_w slice -> [2048, 1024]
All matmuls run as float32r (f32 storage bitcast; full PE rate at N>=256).
"""

import numpy as np
from contextlib import ExitStack

import concourse.bacc as bacc
import concourse.bass as bass
import concourse.tile as tile
from concourse import mybir

P = 128
S = 2048
D = 1024
N_HEADS_TOT = 16
HEADS = 4            # per core
HD = 64
M_DIM = HEADS * HD   # 256
KC = 8               # embed-dim 128-chunks
QCW = 512            # q chunk width
NQC = S // QCW       # 4
NKT = S // P         # 16 k-tiles
F32 = mybir.dt.float32
F32R = mybir.dt.float32r
BF16 = mybir.dt.bfloat16
EXPF = mybir.ActivationFunctionType.Exp
NEG = -1.0e9

TRACE = False
LAST_RESULTS = None
_NC_CACHE = {}


def build_nc(mode: str, compile_: bool = True, probes: bool = False,
             has_bias: bool = False) -> bass.Bass:
    """mode in {causal, nomask, generic}"""
    nc = bacc.Bacc("TRN2", target_bir_lowering=False, debug=False)
    prb = {}
    if probes:
        for nm, shape in (("p_qt", [P, S]), ("p_kt", [P, S]),
                          ("p_va", [P, HEADS * (HD + 1)]), ("p_ct", [P, S])):
            prb[nm] = nc.dram_tensor(nm, shape, F32, kind="ExternalOutput").ap()
    xq = nc.dram_tensor("xqT", [D + 1, S], BF16, kind="ExternalInput").ap()
    xk = nc.dram_tensor("xkT", [D + 1, S], BF16, kind="ExternalInput").ap()
    xv = nc.dram_tensor("xvT", [D + 1, S], BF16, kind="ExternalInput").ap()
    wq = nc.dram_tensor("wqT", [D + 1, M_DIM], BF16, kind="ExternalInput").ap()
    wk = nc.dram_tensor("wkT", [D + 1, M_DIM], BF16, kind="ExternalInput").ap()
    wv = nc.dram_tensor("wvT", [D + 1, M_DIM], BF16, kind="ExternalInput").ap()
    ow = nc.dram_tensor("owT", [M_DIM, D], BF16, kind="ExternalInput").ap()
    btri = nc.dram_tensor("btri", [P, P], F32, kind="ExternalInput").ap()
    bfull = None
    if mode == "generic":
        bfull = nc.dram_tensor("biasT", [S, S], F32, kind="ExternalInput").ap()
    out = nc.dram_tensor("out", [S, D], BF16, kind="ExternalOutput").ap()

    with tile.TileContext(nc) as tc, ExitStack() as ctx:
        consts = ctx.enter_context(tc.tile_pool(name="consts", bufs=1))
        xpool = ctx.enter_context(tc.tile_pool(name="xpool", bufs=48))
        qkv = ctx.enter_context(tc.tile_pool(name="qkv", bufs=1))
        ppool = ctx.enter_context(tc.tile_pool(name="ppool", bufs=4))
        bpool = ctx.enter_context(tc.tile_pool(name="bpool", bufs=2))
        small = ctx.enter_context(tc.tile_pool(name="small", bufs=4))
        outp = ctx.enter_context(tc.tile_pool(name="outp", bufs=2))
        spool = ctx.enter_context(tc.tile_pool(name="spsum", bufs=2, space="PSUM"))
        cpool = ctx.enter_context(tc.tile_pool(name="cpsum", bufs=2, space="PSUM"))
        opool = ctx.enter_context(tc.tile_pool(name="opsum", bufs=2, space="PSUM"))

        # ---- resident weights ----
        def load_w(ap_dram, nm):
            tiles = []
            for kc in range(KC):
                t = consts.tile([P, M_DIM], BF16, name=f"{nm}{kc}")
                nc.sync.dma_start(out=t, in_=ap_dram[P * kc:P * (kc + 1), :])
                tiles.append(t)
            aug = None
            if has_bias:
                aug = consts.tile([1, M_DIM], BF16, name=f"{nm}_aug")
                nc.sync.dma_start(out=aug, in_=ap_dram[D:D + 1, :])
            return tiles, aug

        wq_sb, wq_aug = load_w(wq, "wq")
        wk_sb, wk_aug = load_w(wk, "wk")
        wv_sb, wv_aug = load_w(wv, "wv")
        ow_sb = []
        for pr in range(2):
            t = consts.tile([P, D], BF16, name=f"ow{pr}")
            nc.sync.dma_start(out=t, in_=ow[P * pr:P * (pr + 1), :])
            ow_sb.append(t)
        btri_sb = consts.tile([P, P], F32, name="btri_sb")
        nc.sync.dma_start(out=btri_sb, in_=btri)
        ones4 = consts.tile([P, HEADS], F32, name="ones4")
        nc.vector.memset(ones4, 1.0)

        QT = [qkv.tile([P, S], BF16, name=f"QT{pr}") for pr in range(2)]
        KT = [qkv.tile([P, S], BF16, name=f"KT{pr}") for pr in range(2)]
        CT = [qkv.tile([P, S], BF16, name=f"CT{pr}") for pr in range(2)]
        VA = [qkv.tile([P, HEADS, HD + 1], BF16, name=f"VA{t}") for t in range(NKT)]

        pending_outproj = []

        def emit_outproj(qc):
            for mq in range(QCW // P):
                out_sb = outp.tile([P, D], BF16, name="out_sb")
                q0 = QCW * qc + P * mq
                for ne in range(2):
                    o_ps = opool.tile([P, QCW], F32, name="o_ps")
                    for pr2 in range(2):
                        nc.tensor.matmul(
                            o_ps,
                            lhsT=CT[pr2][:, q0:q0 + P],
                            rhs=ow_sb[pr2][:, QCW * ne:QCW * (ne + 1)],
                            start=(pr2 == 0), stop=(pr2 == 1))
                    nc.vector.tensor_copy(out_sb[:, QCW * ne:QCW * (ne + 1)], o_ps)
                nc.gpsimd.dma_start(out=out[q0:q0 + P, :], in_=out_sb)

        def load_pieces(xap, n):
            """8 [128, 512] pieces of x.T covering q columns of stage n, plus
            the bias ones-row piece."""
            ps = []
            for kc in range(KC):
                xt = xpool.tile([P, QCW], BF16, name="xt")
                nc.sync.dma_start(
                    out=xt,
                    in_=xap[P * kc:P * (kc + 1), QCW * n:QCW * (n + 1)])
                ps.append(xt)
            aug = None
            if has_bias:
                aug = small.tile([1, QCW], BF16, name="xaug", bufs=3)
                nc.sync.dma_start(
                    out=aug, in_=xap[D:D + 1, QCW * n:QCW * (n + 1)])
            return ps, aug

        for n in range(NQC):
            # ---- stage n projections: q/k columns + v rows [512n, 512n+512) ----
            xq_p, xq_a = load_pieces(xq, n)
            xk_p, xk_a = load_pieces(xk, n)
            xv_p, xv_a = load_pieces(xv, n)
            for (x_p, x_a, w_sb, w_aug, dest) in (
                    (xq_p, xq_a, wq_sb, wq_aug, QT),
                    (xk_p, xk_a, wk_sb, wk_aug, KT)):
                for m in range(2):
                    ps = spool.tile([P, 2, QCW], F32, name="s_ps")
                    for kc in range(KC):
                        nc.tensor.matmul(
                            ps[:, 0, :],
                            lhsT=w_sb[kc][:, P * m:P * (m + 1)],
                            rhs=x_p[kc],
                            start=(kc == 0),
                            stop=(not has_bias and kc == KC - 1))
                    if has_bias:
                        nc.tensor.matmul(
                            ps[:, 0, :],
                            lhsT=w_aug[0:1, P * m:P * (m + 1)],
                            rhs=x_a,
                            start=False, stop=True)
                    nc.vector.tensor_copy(
                        dest[m][:, QCW * n:QCW * (n + 1)], ps[:, 0, :])
            for mv in range(4):
                m = 4 * n + mv
                ps = spool.tile([P, 2, QCW], F32, name="s_ps")
                for kc in range(KC):
                    nc.tensor.matmul(
                        ps[:, 0, 0:M_DIM],
                        lhsT=xv_p[kc][:, P * mv:P * (mv + 1)],
                        rhs=wv_sb[kc],
                        start=(kc == 0),
                        stop=(not has_bias and kc == KC - 1))
                if has_bias:
                    nc.tensor.matmul(
                        ps[:, 0, 0:M_DIM],
                        lhsT=xv_a[0:1, P * mv:P * (mv + 1)],
                        rhs=wv_aug,
                        start=False, stop=True)
                nc.vector.tensor_copy(
                    VA[m][:, :, 0:HD],
                    ps[:, 0, 0:M_DIM].rearrange("p (h d) -> p h d", h=HEADS))
                nc.vector.tensor_copy(
                    VA[m][:, :, HD:HD + 1],
                    ones4.rearrange("p (h o) -> p h o", o=1))
            if pending_outproj:
                emit_outproj(pending_outproj.pop(0))
            if probes and n == NQC - 1:
                nc.sync.dma_start(out=prb["p_qt"].bitcast(BF16)[:, 0:S], in_=QT[0])
                nc.sync.dma_start(out=prb["p_kt"].bitcast(BF16)[:, 0:S], in_=KT[0])
                nc.sync.dma_start(
                    out=prb["p_va"].bitcast(BF16)[:, 0:HEADS * (HD + 1)],
                    in_=VA[0].rearrange("p h d -> p (h d)"))

            # ---- stage n attention (q chunk n) ----
            qc = n
            for pr in range(2):
                nt = 4 * qc + 4 if mode == "causal" else NKT
                ctxs = [cpool.tile([HD + 1, QCW], F32, name="ctx_ps")
                        for _ in range(2)]
                queues = ([], [])

                def flush_ctx(j):
                    t0, p0, o0 = queues[j].pop(0)
                    nc.tensor.matmul(
                        ctxs[j][:, o0:],
                        lhsT=VA[t0][:, 2 * pr + j, :],
                        rhs=p0[:, j, o0:],
                        start=(t0 == 0), stop=(t0 == nt - 1),
                        skip_group_check=True)

                for t in range(nt):
                    o = max(0, P * t - QCW * qc) if mode == "causal" else 0
                    s_ps = spool.tile([P, 2, QCW], F32, name="s_ps")
                    for j in range(2):
                        nc.tensor.matmul(
                            s_ps[:, j, o:],
                            lhsT=KT[pr][HD * j:HD * (j + 1), P * t:P * (t + 1)],
                            rhs=QT[pr][HD * j:HD * (j + 1),
                                       QCW * qc + o:QCW * (qc + 1)],
                            start=True, stop=True,
                            tile_position=(HD * j, 0))
                    if mode == "causal" and t >= 4 * qc:
                        nc.vector.tensor_add(
                            s_ps[:, :, o:o + P],
                            s_ps[:, :, o:o + P],
                            btri_sb.rearrange("p (a q) -> p a q", a=1)
                            .to_broadcast([P, 2, P]))
                    elif mode == "generic":
                        bt = bpool.tile([P, QCW], F32, name="bt")
                        nc.sync.dma_start(
                            out=bt,
                            in_=bfull[P * t:P * (t + 1), QCW * qc:QCW * (qc + 1)])
                        nc.vector.tensor_add(
                            s_ps, s_ps,
                            bt.rearrange("p (a q) -> p a q", a=1)
                            .to_broadcast([P, 2, QCW]))
                    p_sb = ppool.tile([P, 2, QCW], BF16, name="p_sb")
                    nc.scalar.activation(
                        p_sb[:, :, o:], s_ps[:, :, o:], EXPF, scale=0.125)
                    for j in range(2):
                        queues[j].append((t, p_sb, o))
                    for j in range(2):
                        if len(queues[j]) > 1:
                            flush_ctx(j)
                for j in range(2):
                    while queues[j]:
                        flush_ctx(j)
                for j in range(2):
                    ctx_ps = ctxs[j]
                    l_sb = small.tile([1, QCW], F32, name="l_sb", bufs=3)
                    nc.vector.tensor_copy(l_sb, ctx_ps[HD:HD + 1, :])
                    r_sb = small.tile([1, QCW], F32, name="r_sb", bufs=3)
                    nc.vector.reciprocal_approx_fast(out=r_sb, in_=l_sb)
                    rbc = ppool.tile([HD, QCW], F32, name="rbc", bufs=2)
                    nc.gpsimd.partition_broadcast(out_ap=rbc, in_ap=r_sb)
                    nc.vector.tensor_mul(
                        CT[pr][HD * j:HD * (j + 1), QCW * qc:QCW * (qc + 1)],
                        ctx_ps[0:HD, :], rbc)

            pending_outproj.append(qc)
        emit_outproj(pending_outproj.pop(0))
        if probes:
            nc.sync.dma_start(out=prb["p_ct"].bitcast(BF16)[:, 0:S], in_=CT[0])

    if compile_:
        nc.compile()
    return nc


def _get_nc(mode, has_bias):
    key = (mode, has_bias)
    if key not in _NC_CACHE:
        _NC_CACHE[key] = build_nc(mode, has_bias=has_bias)
    return _NC_CACHE[key]


def _tri_bias():
    g = np.arange(P, dtype=np.int64)
    return np.where(g[None, :] < g[:, None], np.float32(NEG), np.float32(0.0))


def host_prep(query, key, value, attn_mask, q_w, q_b, k_w, k_b, v_w, v_b, o_w, o_b):
    """Build (mode, in_maps) for the 8 cores."""
    mask = np.asarray(attn_mask).astype(bool)
    if np.array_equal(mask, np.triu(np.ones((S, S), bool), 1)):
        mode = "causal"
    elif not mask.any():
        mode = "nomask"
    else:
        mode = "generic"

    import ml_dtypes
    bf16 = ml_dtypes.bfloat16
    ones_row = np.ones((1, S), bf16)

    def prep_x(x):
        return np.vstack([np.ascontiguousarray(x.T).astype(bf16), ones_row])

    xs = {}
    for b in range(2):
        xs[b] = (prep_x(np.asarray(query)[b]), prep_x(np.asarray(key)[b]),
                 prep_x(np.asarray(value)[b]))

    tri = _tri_bias()
    biasT = None
    if mode == "generic":
        biasT = np.ascontiguousarray(
            np.where(mask, np.float32(NEG), np.float32(0.0)).T)

    def prep_w(w, bvec, sl):
        return np.vstack([
            np.ascontiguousarray(np.asarray(w)[sl].T).astype(bf16),
            np.asarray(bvec)[sl][None, :].astype(bf16)])

    in_maps = []
    for c in range(8):
        b, g = divmod(c, 4)
        sl = slice(M_DIM * g, M_DIM * (g + 1))
        m = {
            "xqT": xs[b][0], "xkT": xs[b][1], "xvT": xs[b][2],
            "wqT": prep_w(q_w, q_b, sl),
            "wkT": prep_w(k_w, k_b, sl),
            "wvT": prep_w(v_w, v_b, sl),
            "owT": np.ascontiguousarray(np.asarray(o_w)[:, sl].T).astype(bf16),
            "btri": tri,
        }
        if mode == "generic":
            m["biasT"] = biasT
        in_maps.append(m)
    return mode, in_maps


def kernel(**inputs) -> np.ndarray:
    global LAST_RESULTS
    from concourse.bass_utils import run_bass_kernel_spmd

    mode, in_maps = host_prep(**inputs)
    has_bias = any(
        np.asarray(inputs[k]).any() for k in ("q_b", "k_b", "v_b"))
    nc = _get_nc(mode, has_bias)
    res = run_bass_kernel_spmd(nc, in_maps, core_ids=list(range(8)), trace=TRACE)
    LAST_RESULTS = res
    parts = [np.asarray(res.results[c]["out"]).astype(np.float32)
             for c in range(8)]
    o_b = np.asarray(inputs["o_b"]).astype(np.float32)
    out = np.stack([
        parts[0] + parts[1] + parts[2] + parts[3],
        parts[4] + parts[5] + parts[6] + parts[7],
    ], axis=0) + o_b[None, None, :]
    return out.astype(np.float32)
